# revision 1
# baseline (speedup 1.0000x reference)
"""Multi-head attention (B=4, T=2048, D=768, H=12) on 8 NeuronCores.

Sharding: core c handles batch b = c//2 and head-group g = c%2 (heads
6g..6g+5).  Each core computes its 6 heads' attention and a partial
output projection (contraction over its 384 local dims of w_proj).  The
host sums the two partials per batch and adds the bias terms.

Device-side formulation (everything transposed so the contraction dim
always lands on SBUF partitions):
  xT   [768, 2048]  (host pre-transposes x[b])
  qT   = Wq_loc.T @ xT   [384, 2048]   (scaled by 1/sqrt(hd), +bias)
  kT   = Wk_loc.T @ xT   [384, 2048]   (+bias)
  v    = x @ Wv_loc      [2048, 384]   (normal layout, no bias; the
         v-bias contributes a constant row handled on host)
  S^T  = kT_h.T @ qT_h   [kpos, q]  per head
  P^T  = exp(S^T)        (scores max ~8 -> no max subtraction needed)
  O'^T = [v_h | 1]^T @ P^T  [65, q]  accumulated over kpos tiles;
         row 64 = softmax denominators
  O^T  normalized via E-matmul broadcast of reciprocal denominators
  y    = O_loc @ Wp_loc  [2048, 768]  partial (host adds partner core)

Schedule: input DMAs are chunked so PE starts within a few us; the V
projection is fused per-kpos-tile into head 0's attention sweep; the
QKV projection of pair p+1 fills PE gaps while ACT paces attention of
pair p; the dt<2 half of the output projection runs during the last
head pair's attention.
"""

import numpy as np

EMBED = 768
HEADS = 12
HD = 64
SCALE = HD ** -0.5
B, T = 4, 2048
NCORES = 8
HPC = 6            # heads per core
DL = HPC * HD      # 384 local model dims per core
USE_FP32R = True

_prog_cache = {}


def _build_program(repeat=1):
    import concourse.bass as bass
    import concourse.mybir as mybir
    import concourse.tile as tile
    from concourse import bacc

    f32 = mybir.dt.float32
    f32r = mybir.dt.float32r
    ACT_EXP = mybir.ActivationFunctionType.Exp

    fm = f32r if USE_FP32R else f32   # storage dtype of matmul operands

    nc = bacc.Bacc()

    xt_d = nc.dram_tensor("xt", [EMBED, T], fm, kind="ExternalInput")
    wq_d = nc.dram_tensor("wq", [EMBED, DL], fm, kind="ExternalInput")
    wk_d = nc.dram_tensor("wk", [EMBED, DL], fm, kind="ExternalInput")
    wv_d = nc.dram_tensor("wv", [EMBED, DL], fm, kind="ExternalInput")
    bqs_d = nc.dram_tensor("bqs", [DL], f32, kind="ExternalInput")
    bk_d = nc.dram_tensor("bk", [DL], f32, kind="ExternalInput")
    wp_d = nc.dram_tensor("wp", [DL, EMBED], fm, kind="ExternalInput")
    e2_d = nc.dram_tensor("e2", [2, 128], fm, kind="ExternalInput")
    y_d = nc.dram_tensor("y", [T, EMBED], f32, kind="ExternalOutput")

    NDT = EMBED // 128   # 6 contraction tiles over embed dim
    NKT = T // 128       # 16 key-position tiles
    NQT = T // 128       # 16 query row tiles
    QH = 2               # process queries in halves of 1024
    QHW = T // QH        # 1024

    with tile.TileContext(nc) as tc:
      for _rep in range(repeat):
        with tc.tile_pool(name="persist", bufs=1) as pers, \
             tc.tile_pool(name="qk", bufs=2) as qk_pool, \
             tc.tile_pool(name="r6p", bufs=2) as r6_pool, \
             tc.tile_pool(name="ps", bufs=2, space="PSUM") as ps_pool, \
             tc.tile_pool(name="ps_s", bufs=2, space="PSUM") as pss_pool, \
             tc.tile_pool(name="ps_o", bufs=2, space="PSUM") as pso_pool, \
             tc.tile_pool(name="pT", bufs=3) as pT_pool:
            v_sb = pers.tile([128, NKT, HPC, HD + 1], fm, name="v_sb")
            oT_sb = pers.tile([128, 3, T], fm, name="oT_sb")
            e2_sb = pers.tile([2, 128], fm, name="e2_sb")
            bqs_sb = pers.tile([128, 3], f32, name="bqs_sb")
            bk_sb = pers.tile([128, 3], f32, name="bk_sb")

            nc.sync.dma_start(out=e2_sb, in_=e2_d.ap())
            nc.gpsimd.dma_start(out=bqs_sb, in_=bqs_d.ap().rearrange("(n p) -> p n", p=128))
            nc.gpsimd.dma_start(out=bk_sb, in_=bk_d.ap().rearrange("(n p) -> p n", p=128))

            # ones column of v' (softmax denominator accumulator): fill the
            # whole tile with 1.0; the value copies below overwrite cols
            # 0:64 of each head slot, leaving col 64 = 1.0
            nc.gpsimd.memset(v_sb.bitcast(f32), 1.0)

            # PE warm-up while the input DMAs stream: dependency-free
            # matmuls on e2 ramp the PE power state before real work lands
            warm_sb = pers.tile([128, 512], fm, name="warm_sb")
            nc.vector.memset(warm_sb.bitcast(f32), 0.0)
            for wi in range(16):
                psw = ps_pool.tile([128, 512], f32, name="psw", tag="ps")
                nc.tensor.matmul(psw, warm_sb[0:2, 0:128], warm_sb[0:2, :],
                                 start=True, stop=True)

            qk_tiles = {}
            r6_tiles = {}

            def proj_qk_chunk(hp, ch, xt_sb, wq_sb, wk_sb):
                qTp, kTp = qk_tiles[hp]
                if True:
                    csl = bass.ts(ch, 512)
                    psq = ps_pool.tile([128, 512], f32, name="psq", tag="ps")
                    psk = ps_pool.tile([128, 512], f32, name="psk", tag="ps")
                    for dt in range(NDT):
                        nc.tensor.matmul(
                            psq,
                            wq_sb[:, dt, bass.ts(hp, 128)],
                            xt_sb[:, dt, csl],
                            start=(dt == 0), stop=(dt == NDT - 1),
                        )
                    for dt in range(NDT):
                        nc.tensor.matmul(
                            psk,
                            wk_sb[:, dt, bass.ts(hp, 128)],
                            xt_sb[:, dt, csl],
                            start=(dt == 0), stop=(dt == NDT - 1),
                        )
                    nc.vector.tensor_scalar(
                        out=qTp[:, csl], in0=psq,
                        scalar1=bqs_sb[:, hp:hp + 1], scalar2=float(SCALE),
                        op0=mybir.AluOpType.add, op1=mybir.AluOpType.mult,
                    )
                    nc.vector.tensor_scalar_add(
                        out=kTp[:, csl], in0=psk,
                        scalar1=bk_sb[:, hp:hp + 1],
                    )

            def proj_qk(hp, xt_sb, wq_sb, wk_sb):
                # qT/kT for head pair hp ([128, T] each, 2 heads stacked)
                qTp = qk_pool.tile([128, T], fm, name="qTp", tag="qT")
                kTp = qk_pool.tile([128, T], fm, name="kTp", tag="kT")
                qk_tiles[hp] = (qTp, kTp)
                for ch in range(4):
                    proj_qk_chunk(hp, ch, xt_sb, wq_sb, wk_sb)

            def emit_v(kt, xt_sb, wv_sb):
                # v (normal layout) for all 6 heads at kpos tile kt
                psv = ps_pool.tile([128, DL], f32, name="psv", tag="ps")
                for dt in range(NDT):
                    nc.tensor.matmul(
                        psv,
                        xt_sb[:, dt, bass.ts(kt, 128)],
                        wv_sb[:, dt, :],
                        start=(dt == 0), stop=(dt == NDT - 1),
                    )
                nc.vector.tensor_copy(
                    out=v_sb[:, kt, :, 0:HD],
                    in_=psv.rearrange("p (h d) -> p h d", h=HPC),
                )

            def attend(h, fuse_v=None, qhs=None):
                # one head: S^T -> exp -> O'^T, denominators to r6.
                # fuse_v: (xt_sb, wv_sb) to emit the V projection per kt
                # during the qh==0 sweep.
                hp, off = h // 2, (h % 2) * 64
                qTp, kTp = qk_tiles[hp]
                if h % 2 == 0 and hp not in r6_tiles:
                    r6_tiles[hp] = r6_pool.tile([2, T], fm, name="r6p", tag="r6")
                r6p = r6_tiles[hp]
                NC2 = QHW // 512
                for qh in (range(QH) if qhs is None else qhs):
                    psos = [
                        pso_pool.tile([65, 512], f32, name="pso", tag="pso")
                        for _ in range(NC2)
                    ]
                    for kt in range(NKT):
                        if fuse_v is not None and qh == 0:
                            emit_v(kt, *fuse_v)
                        pss = pss_pool.tile([128, QHW], f32, name="pss", tag="pss")
                        pT = pT_pool.tile([128, QHW], fm, name="pT", tag="pT")
                        for c2 in range(NC2):
                            nc.tensor.matmul(
                                pss[:, bass.ts(c2, 512)],
                                kTp[off:off + 64, bass.ts(kt, 128)],
                                qTp[off:off + 64, bass.ds(qh * QHW + c2 * 512, 512)],
                                start=True, stop=True,
                            )
                        nc.scalar.activation(out=pT, in_=pss, func=ACT_EXP)
                        for c2 in range(NC2):
                            nc.tensor.matmul(
                                psos[c2],
                                v_sb[:, kt, h, :],
                                pT[:, bass.ts(c2, 512)],
                                start=(kt == 0), stop=(kt == NKT - 1),
                            )
                    for c2 in range(NC2):
                        qssl = bass.ds(qh * QHW + c2 * 512, 512)
                        # engine ops need partition base in {0,32,64,96}:
                        # reciprocal at partition 64, then DMA the row down
                        # to r6's row for this head (DMA has no such limit)
                        rcp_sb = pT_pool.tile([65, 512], fm, name="rcp_sb", tag="rcp", bufs=2)
                        with nc.allow_low_precision(reason="fp32r storage"):
                            nc.vector.reciprocal(
                                out=rcp_sb[64:65, :], in_=psos[c2][64:65, :],
                            )
                        nc.sync.dma_start(
                            out=r6p[h % 2:h % 2 + 1, qssl], in_=rcp_sb[64:65, :],
                        )
                        nc.vector.tensor_copy(
                            out=oT_sb[off:off + 64, hp, qssl], in_=psos[c2][0:64, :],
                        )

            def normalize(hp):
                # oT[:, hp] *= broadcast(1/denom) via the E matmul,
                # chunked so psr rides the pss psum slots (no extra banks)
                r6p = r6_tiles[hp]
                for ch in range(4):
                    csl = bass.ts(ch, 512)
                    psr = ps_pool.tile([128, 512], f32, name="psr", tag="ps")
                    nc.tensor.matmul(
                        psr, e2_sb, r6p[:, csl],
                        start=True, stop=True,
                    )
                    nc.vector.tensor_mul(
                        out=oT_sb[:, hp, csl], in0=oT_sb[:, hp, csl], in1=psr,
                    )

            with tc.tile_pool(name="xw", bufs=1) as xw:
                xt_sb = xw.tile([128, NDT, T], fm, name="xt_sb")
                wq_sb = xw.tile([128, NDT, DL], fm, name="wq_sb")
                wk_sb = xw.tile([128, NDT, DL], fm, name="wk_sb")
                wv_sb = xw.tile([128, NDT, DL], fm, name="wv_sb")

                # chunked input DMAs: xt on the HWDGE queue, weights on the
                # SWDGE queue so they don't serialize behind xt
                nc.gpsimd.dma_start(out=wq_sb, in_=wq_d.ap().rearrange("(n p) m -> p n m", p=128))
                nc.gpsimd.dma_start(out=wk_sb, in_=wk_d.ap().rearrange("(n p) m -> p n m", p=128))
                for dt in range(NDT):
                    nc.sync.dma_start(
                        out=xt_sb[:, dt, :], in_=xt_d.ap()[bass.ts(dt, 128), :],
                    )
                nc.gpsimd.dma_start(out=wv_sb, in_=wv_d.ap().rearrange("(n p) m -> p n m", p=128))

                # startup: interleave qk-pair-0 chunks with head-0/qh0
                # attention steps (S needs only k-chunk kt//4 and q-chunk 0)
                qTp0 = qk_pool.tile([128, T], fm, name="qTp0", tag="qT")
                kTp0 = qk_pool.tile([128, T], fm, name="kTp0", tag="kT")
                qk_tiles[0] = (qTp0, kTp0)
                r6_tiles[0] = r6_pool.tile([2, T], fm, name="r6p0", tag="r6")
                pso0s = [
                    pso_pool.tile([65, 512], f32, name="pso0", tag="pso")
                    for _ in range(2)
                ]
                proj_qk_chunk(0, 0, xt_sb, wq_sb, wk_sb)
                for ch in range(1, 4):
                    proj_qk_chunk(0, ch, xt_sb, wq_sb, wk_sb)
                    for kt in range(4 * (ch - 1), 4 * (ch - 1) + (8 if ch == 3 else 4)):
                        emit_v(kt, xt_sb, wv_sb)
                        pss = pss_pool.tile([128, QHW], f32, name="pss", tag="pss")
                        pT = pT_pool.tile([128, QHW], fm, name="pT", tag="pT")
                        for c2 in range(2):
                            nc.tensor.matmul(
                                pss[:, bass.ts(c2, 512)],
                                kTp0[0:64, bass.ts(kt, 128)],
                                qTp0[0:64, bass.ts(c2, 512)],
                                start=True, stop=True,
                            )
                        nc.scalar.activation(out=pT, in_=pss, func=ACT_EXP)
                        for c2 in range(2):
                            nc.tensor.matmul(
                                pso0s[c2],
                                v_sb[:, kt, 0, :],
                                pT[:, bass.ts(c2, 512)],
                                start=(kt == 0), stop=(kt == NKT - 1),
                            )
                for c2 in range(2):
                    rcp_sb = pT_pool.tile([65, 512], fm, name="rcp_sb", tag="rcp", bufs=2)
                    with nc.allow_low_precision(reason="fp32r storage"):
                        nc.vector.reciprocal(out=rcp_sb[64:65, :], in_=pso0s[c2][64:65, :])
                    nc.sync.dma_start(
                        out=r6_tiles[0][0:1, bass.ts(c2, 512)], in_=rcp_sb[64:65, :])
                    nc.vector.tensor_copy(
                        out=oT_sb[0:64, 0, bass.ts(c2, 512)], in_=pso0s[c2][0:64, :])
                attend(0, qhs=[1])
                proj_qk(1, xt_sb, wq_sb, wk_sb)
                attend(1)
                normalize(0)
                for _q in range(QH):
                    attend(2, qhs=[_q])
                    attend(3, qhs=[_q])
                proj_qk(2, xt_sb, wq_sb, wk_sb)
                normalize(1)

            # x / qkv weights released: run the dt<2 part of the output
            # projection under the last pair's attention
            with tc.tile_pool(name="y01", bufs=1) as y01p, \
                 tc.tile_pool(name="yp", bufs=2) as ypool:
                wp_sb = ypool.tile([128, 3, EMBED], fm, name="wp_sb", bufs=1)
                nc.sync.dma_start(out=wp_sb, in_=wp_d.ap().rearrange("(n p) m -> p n m", p=128))
                y01_sb = y01p.tile([128, NQT, EMBED], f32, name="y01_sb")

                def proj01(qts):
                    for qt in qts:
                        for nh in range(2):
                            psy = ps_pool.tile([128, 512], f32, name="psy", tag="ps")
                            for dt in range(2):
                                nc.tensor.matmul(
                                    psy[:, 0:384],
                                    oT_sb[:, dt, bass.ts(qt, 128)],
                                    wp_sb[:, dt, bass.ts(nh, 384)],
                                    start=(dt == 0), stop=(dt == 1),
                                )
                            nc.vector.tensor_copy(
                                out=y01_sb[:, qt, bass.ts(nh, 384)],
                                in_=psy[:, 0:384],
                            )

                for _q in range(QH):
                    attend(4, qhs=[_q])
                    attend(5, qhs=[_q])
                    proj01(range(NQT // QH * _q, NQT // QH * (_q + 1)))

                # tail: per 512-wide chunk, normalize pair 2 then finish the
                # dt=2 projection in place and ship the output chunk
                r6p2 = r6_tiles[2]
                for c in range(4):
                    csl = bass.ts(c, 512)
                    psr = ps_pool.tile([128, 512], f32, name="psr", tag="ps")
                    nc.tensor.matmul(psr, e2_sb, r6p2[:, csl], start=True, stop=True)
                    nc.vector.tensor_mul(
                        out=oT_sb[:, 2, csl], in0=oT_sb[:, 2, csl], in1=psr,
                    )
                    for qt in range(4 * c, 4 * c + 4):
                        for nh in range(2):
                            psy2 = ps_pool.tile([128, 512], f32, name="psy2", tag="ps")
                            nc.tensor.matmul(
                                psy2[:, 0:384],
                                oT_sb[:, 2, bass.ts(qt, 128)],
                                wp_sb[:, 2, bass.ts(nh, 384)],
                                start=True, stop=True,
                            )
                            nc.vector.tensor_add(
                                out=y01_sb[:, qt, bass.ts(nh, 384)],
                                in0=y01_sb[:, qt, bass.ts(nh, 384)],
                                in1=psy2[:, 0:384],
                            )
                    for c2 in range(4):
                        cc = 4 * c + c2
                        nc.sync.dma_start(
                            out=y_d.ap()[bass.ds(128 * cc, 128), :],
                            in_=y01_sb[:, cc, :],
                        )

    nc.finalize()
    return nc


def _shard_inputs(x, w_qkv, b_qkv, w_proj):
    e2 = np.zeros((2, 128), dtype=np.float32)
    e2[0, 0:HD] = 1.0
    e2[1, HD:128] = 1.0
    in_maps = []
    for c in range(NCORES):
        b, g = c // 2, c % 2
        sl = slice(DL * g, DL * g + DL)
        in_maps.append({
            "xt": np.ascontiguousarray(x[b].T),
            "wq": np.ascontiguousarray(w_qkv[:, sl]),
            "wk": np.ascontiguousarray(w_qkv[:, EMBED:][:, sl]),
            "wv": np.ascontiguousarray(w_qkv[:, 2 * EMBED:][:, sl]),
            "bqs": np.ascontiguousarray(b_qkv[sl]),
            "bk": np.ascontiguousarray(b_qkv[EMBED:][sl]),
            "wp": np.ascontiguousarray(w_proj[sl, :]),
            "e2": e2,
        })
    return in_maps


def kernel(x, w_qkv, b_qkv, w_proj, b_proj, _profile=False, _repeat=1):
    from concourse.bass_utils import run_bass_kernel_spmd

    x = np.asarray(x, dtype=np.float32)
    w_qkv = np.asarray(w_qkv, dtype=np.float32)
    b_qkv = np.asarray(b_qkv, dtype=np.float32)
    w_proj = np.asarray(w_proj, dtype=np.float32)
    b_proj = np.asarray(b_proj, dtype=np.float32)

    if _repeat not in _prog_cache:
        _prog_cache[_repeat] = _build_program(_repeat)
    nc = _prog_cache[_repeat]

    in_maps = _shard_inputs(x, w_qkv, b_qkv, w_proj)
    res = run_bass_kernel_spmd(
        nc, in_maps, list(range(NCORES)), trace=_profile,
    )

    # host-side gather: sum the two head-group partials per batch and add
    # the bias row (v-bias folded through w_proj, plus b_proj itself)
    bias_row = b_qkv[2 * EMBED:] @ w_proj + b_proj
    y = np.empty((B, T, EMBED), dtype=np.float32)
    for b in range(B):
        y[b] = res.results[2 * b]["y"] + res.results[2 * b + 1]["y"] + bias_row
    if _profile:
        return y, res
    return y



# revision 23
# speedup vs baseline: 1.1555x; 1.1555x over previous
"""Multi-head attention (B=4, T=2048, D=768, H=12) on 8 NeuronCores.

Sharding: core c handles batch b = c//2 and head-group g = c%2 (heads
6g..6g+5).  Each core computes its 6 heads' attention and a partial
output projection (contraction over its 384 local dims of w_proj); the
host sums the two partials per batch and adds the bias row.

Device formulation (bf16 matmul operands everywhere, fp32 psum):
  qT = Wq'.T @ xT  [384, 2048]   (Wq' pre-scaled by 1/sqrt(hd) on host)
  kT = Wk.T @ xT   [384, 2048]
  v  = x @ Wv      [2048, 384]   per kpos tile (65th column = 1.0)
  S^T[kt] = kT_h.T @ qT_h   [128 kpos, 1024 q]  per head, q-slab halves
  P^T = exp(S^T)   (ACT; scores max ~8 so no max subtraction)
  O[q, 65] += P^T[kt].T @ v'[kt]   <- flipped: output partitions = 128 q
      (col 64 accumulates the softmax denominators via the ones column)
  o = O[:, 0:64] * rcp(O[:, 64])   fused normalize in the psum drain
  oT via PE transpose (identity stationary), y = oT.T @ Wp per q tile.

The flip halves the P@V matmul cost vs the [65, 512]-output orientation
(the timeline cost model charges out-free-size cycles per matmul, so
output partition utilization is what matters).  Schedule: ACT (exp)
paces the attention inner loop at ~1.04us per [128,1024] tile; S, O,
QKV-projection, V, transposes and the output projection are spread
across the 12 (head, q-slab) sweeps to keep PE under that pace.
O-matmuls trail their exp by 4 kt iterations so psum-slot drains (DVE)
never stall the PE queue head.
"""

import numpy as np

EMBED = 768
HEADS = 12
HD = 64
SCALE = HD ** -0.5
B, T = 4, 2048
NCORES = 8
HPC = 6            # heads per core
DL = HPC * HD      # 384 local model dims per core

_prog_cache = {}


def _build_program(repeat=1):
    import concourse.bass as bass
    import concourse.mybir as mybir
    import concourse.tile as tile
    from concourse import bacc

    f32 = mybir.dt.float32
    bf16 = mybir.dt.bfloat16
    ACT_EXP = mybir.ActivationFunctionType.Exp
    ACT_COPY = mybir.ActivationFunctionType.Copy

    nc = bacc.Bacc()

    xt_d = nc.dram_tensor("xt", [EMBED, T], bf16, kind="ExternalInput")
    wq_d = nc.dram_tensor("wq", [EMBED, DL], bf16, kind="ExternalInput")
    wk_d = nc.dram_tensor("wk", [EMBED, DL], bf16, kind="ExternalInput")
    wv_d = nc.dram_tensor("wv", [EMBED, DL], bf16, kind="ExternalInput")
    bqs_d = nc.dram_tensor("bqs", [DL], f32, kind="ExternalInput")
    bk_d = nc.dram_tensor("bk", [DL], f32, kind="ExternalInput")
    wp_d = nc.dram_tensor("wp", [DL, EMBED], bf16, kind="ExternalInput")
    y_d = nc.dram_tensor("y", [T, EMBED], f32, kind="ExternalOutput")

    NDT = EMBED // 128   # 6 contraction tiles over embed dim
    NKT = T // 128       # 16 key-position tiles
    NQT = T // 128       # 16 query row tiles
    LAG = 4              # O-matmul lag (in kt iterations) behind exp

    # (head, q-slab) sweep order: q-major within each head pair so a
    # pair's q-half completes as early as possible (feeds transposes).
    SLABS = [(0, 0), (1, 0), (0, 1), (1, 1),
             (2, 0), (3, 0), (2, 1), (3, 1),
             (4, 0), (5, 0), (4, 1), (5, 1)]

    with tile.TileContext(nc) as tc:
      for _rep in range(repeat):
        with tc.tile_pool(name="pers", bufs=1) as pers, \
             tc.tile_pool(name="qk", bufs=2) as qk_pool, \
             tc.tile_pool(name="pt", bufs=2) as pt_pool, \
             tc.tile_pool(name="rcp", bufs=4) as rcp_pool, \
             tc.tile_pool(name="yr", bufs=3) as yr_pool, \
             tc.tile_pool(name="pss", bufs=2, space="PSUM") as pss_pool, \
             tc.tile_pool(name="po", bufs=2, space="PSUM") as po_pool, \
             tc.tile_pool(name="aux", bufs=2, space="PSUM") as aux_pool:

            xt_dts = [pers.tile([128, T], bf16, name=f"xt{dt}_sb")
                      for dt in range(NDT)]
            wq_sb = pers.tile([128, NDT, DL], bf16, name="wq_sb")
            wk_sb = pers.tile([128, NDT, DL], bf16, name="wk_sb")
            wv_sb = pers.tile([128, NDT, DL], bf16, name="wv_sb")
            wp_sb = pers.tile([128, 3, EMBED], bf16, name="wp_sb")
            v_sb = pers.tile([128, NKT, HPC, HD + 1], bf16, name="v_sb")
            bqs_sb = pers.tile([128, 3], f32, name="bqs_sb")
            bk_sb = pers.tile([128, 3], f32, name="bk_sb")
            o_sb = pers.tile([128, NQT, 3, 128], bf16, name="o_sb")
            oT_sb = pers.tile([128, 3, T], bf16, name="oT_sb")
            warm_sb = pers.tile([128, 256], bf16, name="warm_sb")

            # input DMAs: first-slab critical path is wk + all of xt (full
            # embed contraction), so those go first; per-dt xt tiles give
            # each transfer its own completion sem (DMA write deps are
            # tile × queue granular)
            nc.gpsimd.dma_start(out=wk_sb, in_=wk_d.ap().rearrange("(n p) m -> p n m", p=128))
            nc.gpsimd.dma_start(out=wq_sb, in_=wq_d.ap().rearrange("(n p) m -> p n m", p=128))
            nc.gpsimd.dma_start(out=bqs_sb, in_=bqs_d.ap().rearrange("(n p) -> p n", p=128))
            nc.gpsimd.dma_start(out=bk_sb, in_=bk_d.ap().rearrange("(n p) -> p n", p=128))
            for dt in range(NDT):
                eng = nc.sync if dt % 2 == 0 else nc.gpsimd
                eng.dma_start(out=xt_dts[dt], in_=xt_d.ap()[bass.ts(dt, 128), :])
            nc.gpsimd.dma_start(out=wv_sb, in_=wv_d.ap().rearrange("(n p) m -> p n m", p=128))
            nc.gpsimd.dma_start(out=wp_sb, in_=wp_d.ap().rearrange("(n p) m -> p n m", p=128))

            # ones column of v' (softmax denominator accumulator): fill
            # the whole tile; value copies overwrite cols 0:64 per head
            nc.gpsimd.memset(v_sb, 1.0)
            nc.vector.memset(warm_sb, 0.0)

            # PE warm-up while input DMAs stream: enough matmuls to span the
            # ~12us serial-DMA window so the p-state ramp completes before
            # the first real work
            for wi in range(40):
                psw = aux_pool.tile([128, 256], f32, name="psw", tag="aux")
                nc.tensor.matmul(psw, warm_sb[0:2, 0:128], warm_sb[0:2, :],
                                 start=True, stop=True)

            qk_tiles = {}
            yr_tiles = {}

            def mk_pair(hp):
                qk_tiles[hp] = (
                    qk_pool.tile([128, T], bf16, name="qTp", tag="qT"),
                    qk_pool.tile([128, T], bf16, name="kTp", tag="kT"),
                )

            def qkv_group(hp, ch, which):
                csl = bass.ts(ch, 512)
                qTp, kTp = qk_tiles[hp]
                dst, wsb, bias = (
                    (qTp, wq_sb, bqs_sb) if which == "q" else (kTp, wk_sb, bk_sb)
                )
                ps = aux_pool.tile([128, 512], f32, name="psqk", tag="aux")
                for dt in range(NDT):
                    nc.tensor.matmul(
                        ps, wsb[:, dt, bass.ts(hp, 128)], xt_dts[dt][:, csl],
                        start=(dt == 0), stop=(dt == NDT - 1),
                    )
                nc.vector.tensor_scalar_add(
                    out=dst[:, csl], in0=ps, scalar1=bias[:, hp:hp + 1],
                )

            def v_emit(kt):
                ps = aux_pool.tile([128, DL], f32, name="psv", tag="aux")
                for dt in range(NDT):
                    nc.tensor.matmul(
                        ps, xt_dts[dt][:, bass.ts(kt, 128)], wv_sb[:, dt, :],
                        start=(dt == 0), stop=(dt == NDT - 1),
                    )
                # GPSIMD cannot touch PSUM; ACT is mostly idle during the
                # V-emission slab, so drain there
                nc.scalar.activation(
                    out=v_sb[:, kt, :, 0:HD],
                    in_=ps.rearrange("p (h d) -> p h d", h=HPC),
                    func=ACT_COPY,
                )

            def transpose_qt(pair, qt, tail=False):
                # 2-byte dtypes transpose on the DMA xbar (PE transpose into
                # psum is 4-byte-cell granular and corrupts bf16).  Tail
                # transposes issue from the otherwise-idle ACT queue so their
                # descriptor generation doesn't serialize behind the y DMAs.
                eng = nc.scalar if tail else nc.sync
                eng.dma_start_transpose(
                    out=oT_sb[:, pair, bass.ts(qt, 128)],
                    in_=o_sb[:, qt, pair, :],
                )

            def proj_group(qt, nh, tail=False):
                ps = aux_pool.tile([128, 384], f32, name="psy", tag="aux")
                for dtp in range(3):
                    nc.tensor.matmul(
                        ps, oT_sb[:, dtp, bass.ts(qt, 128)],
                        wp_sb[:, dtp, bass.ts(nh, 384)],
                        start=(dtp == 0), stop=(dtp == 2),
                    )
                if nh == 0:
                    yr = yr_pool.tile([128, EMBED], f32, name="yr", tag="yr")
                    yr_tiles[qt] = yr
                    nc.vector.tensor_copy(out=yr[:, 0:384], in_=ps)
                else:
                    yr = yr_tiles.pop(qt)
                    if tail:  # ACT is idle once attention has drained
                        nc.scalar.activation(out=yr[:, 384:768], in_=ps, func=ACT_COPY)
                    else:
                        nc.vector.tensor_copy(out=yr[:, 384:768], in_=ps)
                    nc.sync.dma_start(out=y_d.ap()[bass.ts(qt, 128), :], in_=yr)

            def drain_po(h, qh, po, qt_base):
                # fused normalize: o = O[:, 0:64] / O[:, 64] at psum drain
                pair, off = h // 2, (h % 2) * HD
                rcp = rcp_pool.tile([128, 4], f32, name="rcp", tag="rcp")
                nc.vector.reciprocal(out=rcp, in_=po[:, :, HD])
                for j in range(4):
                    qt = qh * 8 + qt_base + j
                    nc.vector.tensor_scalar_mul(
                        out=o_sb[:, qt, pair, off:off + HD],
                        in0=po[:, j, 0:HD], scalar1=rcp[:, j:j + 1],
                    )

            spill = []   # closures: previous slab's trailing O-matmuls + drains

            def attend(h, qh, fillers):
                nonlocal spill
                hp, off = h // 2, (h % 2) * HD
                qTp, kTp = qk_tiles[hp]
                pts = pt_pool.tile([128, NKT, 1024], bf16, name="pts", tag="pt")
                po_t = [None, None]
                myspill = []

                def own_o(kt):
                    for qt in range(8):
                        po = po_t[qt // 4]
                        # start=True zeroes the whole 2KB psum bank, so only
                        # the first column of each po bank may assert it
                        nc.tensor.matmul(
                            po[:, qt % 4, :],
                            pts[:, kt, bass.ts(qt, 128)],
                            v_sb[:, kt, h, :],
                            start=(kt == 0 and qt % 4 == 0),
                            stop=(kt == NKT - 1),
                            skip_group_check=True,
                        )

                fi = 0
                # fillers may read tiles written by the previous slab's
                # drains (spill[2..3], emitted at j=2,3) — dependency
                # tracking is emission-ordered, so fillers wait until j=4
                fstart = 4 if spill else 0

                def emit_fillers(j):
                    nonlocal fi
                    if j < fstart:
                        return
                    span = NKT - fstart
                    upto = min(
                        (len(fillers) * (j - fstart + 1) + span - 1) // span,
                        len(fillers),
                    )
                    while fi < upto:
                        fillers[fi]()
                        fi += 1

                for j in range(NKT):
                    pss = pss_pool.tile([128, 1024], f32, name="pss", tag="pss")
                    for c2 in range(2):
                        nc.tensor.matmul(
                            pss[:, bass.ts(c2, 512)],
                            kTp[off:off + HD, bass.ts(j, 128)],
                            qTp[off:off + HD, bass.ds(qh * 1024 + c2 * 512, 512)],
                            start=True, stop=True,
                        )
                    nc.scalar.activation(out=pts[:, j, :], in_=pss, func=ACT_EXP)
                    if j < len(spill):
                        spill[j]()
                    if j == LAG:
                        po_t[0] = po_pool.tile([128, 4, HD + 1], f32, name="po0", tag="po")
                        po_t[1] = po_pool.tile([128, 4, HD + 1], f32, name="po1", tag="po")
                    if j >= LAG:
                        own_o(j - LAG)
                    emit_fillers(j)

                # trailing O-matmuls packed two per iteration so the psum
                # drains land at j=2,3 of the next slab — two iterations
                # before its own O-matmuls reallocate the po slots at j=4
                myspill.append(lambda: (own_o(NKT - 4), own_o(NKT - 3)))
                myspill.append(lambda: (own_o(NKT - 2), own_o(NKT - 1)))
                myspill.append(lambda: drain_po(h, qh, po_t[0], 0))
                myspill.append(lambda: drain_po(h, qh, po_t[1], 4))
                spill = myspill

            # ---- static filler schedule -------------------------------
            mk_pair(0)
            F = {s: [] for s in range(1, 13)}
            F[1] = (
                [lambda kt=kt: v_emit(kt) for kt in range(4)]
                + [lambda: qkv_group(0, 1, "k")]
                + [lambda kt=kt: v_emit(kt) for kt in range(4, 8)]
                + [lambda: qkv_group(0, 2, "k")]
                + [lambda kt=kt: v_emit(kt) for kt in range(8, 12)]
                + [lambda: qkv_group(0, 3, "k")]
                + [lambda kt=kt: v_emit(kt) for kt in range(12, 16)]
            )
            F[2] = [
                lambda: qkv_group(0, 2, "q"),
                lambda: qkv_group(0, 3, "q"),
                lambda: mk_pair(1),
                lambda: qkv_group(1, 0, "k"),
                lambda: qkv_group(1, 0, "q"),
            ]
            F[3] = (
                [lambda: qkv_group(1, 1, "k"),
                 lambda: qkv_group(1, 1, "q"),
                 lambda: qkv_group(1, 2, "k")]
                + [lambda qt=qt: transpose_qt(0, qt) for qt in range(8)]
            )
            F[4] = [
                lambda: qkv_group(1, 2, "q"),
                lambda: qkv_group(1, 3, "k"),
                lambda: qkv_group(1, 3, "q"),
            ]
            F[5] = (
                [lambda: mk_pair(2),
                 lambda: qkv_group(2, 0, "k"),
                 lambda: qkv_group(2, 0, "q")]
                + [lambda qt=qt: transpose_qt(0, qt) for qt in range(8, 16)]
            )
            F[6] = [
                lambda: qkv_group(2, 1, "k"),
                lambda: qkv_group(2, 1, "q"),
            ]
            F[7] = (
                [lambda: qkv_group(2, 2, "k"),
                 lambda: qkv_group(2, 2, "q")]
                + [lambda qt=qt: transpose_qt(1, qt) for qt in range(8)]
            )
            F[8] = [
                lambda: qkv_group(2, 3, "k"),
                lambda: qkv_group(2, 3, "q"),
            ]
            F[9] = [lambda qt=qt: transpose_qt(1, qt) for qt in range(8, 16)]
            F[10] = []
            F[11] = (
                [lambda qt=qt: transpose_qt(2, qt) for qt in range(8)]
                + [lambda qt=qt, nh=nh: proj_group(qt, nh)
                   for qt in range(4) for nh in range(2)]
            )
            F[12] = [lambda qt=qt, nh=nh: proj_group(qt, nh)
                     for qt in range(4, 8) for nh in range(2)]

            # ---- startup: pair-0 chunks needed by the first slab ------
            qkv_group(0, 0, "k")
            qkv_group(0, 0, "q")
            qkv_group(0, 1, "q")

            for s, (h, qh) in enumerate(SLABS, start=1):
                attend(h, qh, F[s])

            # ---- tail -------------------------------------------------
            for fn in spill:
                fn()
            for qt in range(8, 16):
                transpose_qt(2, qt, tail=True)
            for qt in range(8, 16):
                for nh in range(2):
                    proj_group(qt, nh, tail=True)

    nc.finalize()
    return nc


def _shard_inputs(x, w_qkv, b_qkv, w_proj):
    import ml_dtypes

    bf16 = ml_dtypes.bfloat16
    in_maps = []
    for c in range(NCORES):
        b, g = c // 2, c % 2
        sl = slice(DL * g, DL * g + DL)
        in_maps.append({
            "xt": np.ascontiguousarray(x[b].T).astype(bf16),
            "wq": np.ascontiguousarray(w_qkv[:, sl] * SCALE).astype(bf16),
            "wk": np.ascontiguousarray(w_qkv[:, EMBED:][:, sl]).astype(bf16),
            "wv": np.ascontiguousarray(w_qkv[:, 2 * EMBED:][:, sl]).astype(bf16),
            "bqs": np.ascontiguousarray(b_qkv[sl] * SCALE),
            "bk": np.ascontiguousarray(b_qkv[EMBED:][sl]),
            "wp": np.ascontiguousarray(w_proj[sl, :]).astype(bf16),
        })
    return in_maps


def kernel(x, w_qkv, b_qkv, w_proj, b_proj, _profile=False, _repeat=1):
    from concourse.bass_utils import run_bass_kernel_spmd

    x = np.asarray(x, dtype=np.float32)
    w_qkv = np.asarray(w_qkv, dtype=np.float32)
    b_qkv = np.asarray(b_qkv, dtype=np.float32)
    w_proj = np.asarray(w_proj, dtype=np.float32)
    b_proj = np.asarray(b_proj, dtype=np.float32)

    if _repeat not in _prog_cache:
        _prog_cache[_repeat] = _build_program(_repeat)
    nc = _prog_cache[_repeat]

    in_maps = _shard_inputs(x, w_qkv, b_qkv, w_proj)
    res = run_bass_kernel_spmd(
        nc, in_maps, list(range(NCORES)), trace=_profile,
    )

    # host-side gather: sum the two head-group partials per batch and add
    # the bias row (v-bias folded through w_proj, plus b_proj itself)
    bias_row = b_qkv[2 * EMBED:] @ w_proj + b_proj
    y = np.empty((B, T, EMBED), dtype=np.float32)
    for b in range(B):
        y[b] = res.results[2 * b]["y"] + res.results[2 * b + 1]["y"] + bias_row
    if _profile:
        return y, res
    return y


# revision 32
# speedup vs baseline: 1.1759x; 1.0176x over previous
"""Multi-head attention (B=4, T=2048, D=768, H=12) on 8 NeuronCores.

Sharding: core c handles batch b = c//2 and head-group g = c%2 (heads
6g..6g+5).  Each core computes its 6 heads' attention and a partial
output projection (contraction over its 384 local dims of w_proj); the
host sums the two partials per batch and adds the bias row.

Device formulation (bf16 matmul operands everywhere, fp32 psum):
  qT = Wq'.T @ xT  [384, 2048]   (Wq' pre-scaled by 1/sqrt(hd) on host)
  kT = Wk.T @ xT   [384, 2048]
  v  = x @ Wv      [2048, 384]   per kpos tile (65th column = 1.0)
  S^T[kt] = kT_h.T @ qT_h   [128 kpos, 1024 q]  per head, q-slab halves
  P^T = exp(S^T)   (ACT; scores max ~8 so no max subtraction)
  O[q, 65] += P^T[kt].T @ v'[kt]   <- flipped: output partitions = 128 q
      (col 64 accumulates the softmax denominators via the ones column)
  o = O[:, 0:64] * rcp(O[:, 64])   fused normalize in the psum drain
  oT via PE transpose (identity stationary), y = oT.T @ Wp per q tile.

The flip halves the P@V matmul cost vs the [65, 512]-output orientation
(the timeline cost model charges out-free-size cycles per matmul, so
output partition utilization is what matters).  Schedule: ACT (exp)
paces the attention inner loop at ~1.04us per [128,1024] tile; S, O,
QKV-projection, V, transposes and the output projection are spread
across the 12 (head, q-slab) sweeps to keep PE under that pace.
O-matmuls trail their exp by 4 kt iterations so psum-slot drains (DVE)
never stall the PE queue head.
"""

import numpy as np

EMBED = 768
HEADS = 12
HD = 64
SCALE = HD ** -0.5
B, T = 4, 2048
NCORES = 8
HPC = 6            # heads per core
DL = HPC * HD      # 384 local model dims per core

_prog_cache = {}


def _build_program(repeat=1):
    import concourse.bass as bass
    import concourse.mybir as mybir
    import concourse.tile as tile
    from concourse import bacc

    f32 = mybir.dt.float32
    bf16 = mybir.dt.bfloat16
    ACT_EXP = mybir.ActivationFunctionType.Exp
    ACT_COPY = mybir.ActivationFunctionType.Copy

    nc = bacc.Bacc()

    xt_d = nc.dram_tensor("xt", [EMBED, T], bf16, kind="ExternalInput")
    wq_d = nc.dram_tensor("wq", [EMBED, DL], bf16, kind="ExternalInput")
    wk_d = nc.dram_tensor("wk", [EMBED, DL], bf16, kind="ExternalInput")
    wv_d = nc.dram_tensor("wv", [EMBED, DL], bf16, kind="ExternalInput")
    bqs_d = nc.dram_tensor("bqs", [DL], f32, kind="ExternalInput")
    bk_d = nc.dram_tensor("bk", [DL], f32, kind="ExternalInput")
    wp_d = nc.dram_tensor("wp", [DL, EMBED], bf16, kind="ExternalInput")
    y_d = nc.dram_tensor("y", [T, EMBED], bf16, kind="ExternalOutput")

    NDT = EMBED // 128   # 6 contraction tiles over embed dim
    NKT = T // 128       # 16 key-position tiles
    NQT = T // 128       # 16 query row tiles
    LAG = 4              # O-matmul lag (in kt iterations) behind exp

    # (head, q-slab) sweep order: q-major within each head pair so a
    # pair's q-half completes as early as possible (feeds transposes).
    SLABS = [(0, 0), (1, 0), (0, 1), (1, 1),
             (2, 0), (3, 0), (2, 1), (3, 1),
             (4, 0), (5, 0), (4, 1), (5, 1)]

    with tile.TileContext(nc) as tc:
      for _rep in range(repeat):
        with tc.tile_pool(name="pers", bufs=1) as pers, \
             tc.tile_pool(name="qk", bufs=2) as qk_pool, \
             tc.tile_pool(name="pt", bufs=2) as pt_pool, \
             tc.tile_pool(name="rcp", bufs=4) as rcp_pool, \
             tc.tile_pool(name="yr", bufs=3) as yr_pool, \
             tc.tile_pool(name="pss", bufs=2, space="PSUM") as pss_pool, \
             tc.tile_pool(name="po", bufs=2, space="PSUM") as po_pool, \
             tc.tile_pool(name="aux", bufs=2, space="PSUM") as aux_pool:

            xt_dts = [pers.tile([128, T], bf16, name=f"xt{dt}_sb")
                      for dt in range(NDT)]
            wq_sb = pers.tile([128, NDT, DL], bf16, name="wq_sb")
            wk_sb = pers.tile([128, NDT, DL], bf16, name="wk_sb")
            wv_sb = pers.tile([128, NDT, DL], bf16, name="wv_sb")
            wp_sb = pers.tile([128, 3, EMBED], bf16, name="wp_sb")
            v_sb = pers.tile([128, NKT, HPC, HD + 1], bf16, name="v_sb")
            bqs_sb = pers.tile([128, 3], f32, name="bqs_sb")
            bk_sb = pers.tile([128, 3], f32, name="bk_sb")
            o_sb = pers.tile([128, NQT, 3, 128], bf16, name="o_sb")
            oT_sb = pers.tile([128, 3, T], bf16, name="oT_sb")
            warm_sb = pers.tile([128, 256], bf16, name="warm_sb")

            # ones column of v' (softmax denominator accumulator) — only
            # the 65th columns; emitted first so it doesn't sit behind the
            # DMA descriptor generation on the Pool queue
            nc.gpsimd.memset(v_sb[:, :, :, HD:HD + 1], 1.0)
            nc.vector.memset(warm_sb, 0.0)

            # input DMAs: first-slab critical path is wk/wq + all of xt
            # (full embed contraction).  The DMA copies serialize on one
            # resource, so everything not needed before the first S goes
            # after xt.  Per-dt xt tiles give each transfer its own
            # completion sem (DMA write deps are tile x queue granular).
            nc.gpsimd.dma_start(out=wk_sb, in_=wk_d.ap().rearrange("(n p) m -> p n m", p=128))
            nc.gpsimd.dma_start(out=wq_sb, in_=wq_d.ap().rearrange("(n p) m -> p n m", p=128))
            for dt in range(3):
                nc.sync.dma_start(out=xt_dts[dt], in_=xt_d.ap()[bass.ts(dt, 128), :])
            for dt in range(3, NDT):
                nc.gpsimd.dma_start(out=xt_dts[dt], in_=xt_d.ap()[bass.ts(dt, 128), :])
            nc.gpsimd.dma_start(out=wv_sb, in_=wv_d.ap().rearrange("(n p) m -> p n m", p=128))
            nc.gpsimd.dma_start(out=wp_sb, in_=wp_d.ap().rearrange("(n p) m -> p n m", p=128))
            nc.sync.dma_start(out=bqs_sb, in_=bqs_d.ap().rearrange("(n p) -> p n", p=128))
            nc.sync.dma_start(out=bk_sb, in_=bk_d.ap().rearrange("(n p) -> p n", p=128))

            def warm(n):
                # warm-up matmuls ride the po slots (idle until the first
                # slab's O accumulation; pss holds ps_q01 through startup)
                for _w in range(n):
                    psw = po_pool.tile([128, 256], f32, name="psw", tag="po")
                    nc.tensor.matmul(psw, warm_sb[0:2, 0:128], warm_sb[0:2, :],
                                     start=True, stop=True)

            warm(6)

            qk_tiles = {}
            yr_tiles = {}

            def mk_pair(hp):
                qk_tiles[hp] = (
                    qk_pool.tile([128, T], bf16, name="qTp", tag="qT"),
                    qk_pool.tile([128, T], bf16, name="kTp", tag="kT"),
                )

            def qkv_group(hp, ch, which):
                csl = bass.ts(ch, 512)
                qTp, kTp = qk_tiles[hp]
                dst, wsb, bias = (
                    (qTp, wq_sb, bqs_sb) if which == "q" else (kTp, wk_sb, bk_sb)
                )
                ps = aux_pool.tile([128, 512], f32, name="psqk", tag="aux")
                for dt in range(NDT):
                    nc.tensor.matmul(
                        ps, wsb[:, dt, bass.ts(hp, 128)], xt_dts[dt][:, csl],
                        start=(dt == 0), stop=(dt == NDT - 1),
                    )
                nc.vector.tensor_scalar_add(
                    out=dst[:, csl], in0=ps, scalar1=bias[:, hp:hp + 1],
                )

            def v_emit(kt):
                ps = aux_pool.tile([128, DL], f32, name="psv", tag="aux")
                for dt in range(NDT):
                    nc.tensor.matmul(
                        ps, xt_dts[dt][:, bass.ts(kt, 128)], wv_sb[:, dt, :],
                        start=(dt == 0), stop=(dt == NDT - 1),
                    )
                # GPSIMD cannot touch PSUM; DVE is nearly idle during the
                # V-emission slab (ACT copies here would stall its in-order
                # queue ahead of the exps)
                nc.vector.tensor_copy(
                    out=v_sb[:, kt, :, 0:HD],
                    in_=ps.rearrange("p (h d) -> p h d", h=HPC),
                )

            def transpose_qt(pair, qt, tail=False):
                # 2-byte dtypes transpose on the DMA xbar (PE transpose into
                # psum is 4-byte-cell granular and corrupts bf16).  Tail
                # transposes issue from the otherwise-idle ACT queue so their
                # descriptor generation doesn't serialize behind the y DMAs.
                eng = nc.scalar if tail else nc.sync
                eng.dma_start_transpose(
                    out=oT_sb[:, pair, bass.ts(qt, 128)],
                    in_=o_sb[:, qt, pair, :],
                )

            def proj_group(qt, nh, tail=False):
                ps = aux_pool.tile([128, 384], f32, name="psy", tag="aux")
                for dtp in range(3):
                    nc.tensor.matmul(
                        ps, oT_sb[:, dtp, bass.ts(qt, 128)],
                        wp_sb[:, dtp, bass.ts(nh, 384)],
                        start=(dtp == 0), stop=(dtp == 2),
                    )
                if nh == 0:
                    yr = yr_pool.tile([128, EMBED], bf16, name="yr", tag="yr")
                    yr_tiles[qt] = yr
                    nc.vector.tensor_copy(out=yr[:, 0:384], in_=ps)
                else:
                    yr = yr_tiles.pop(qt)
                    if tail:  # ACT is idle once attention has drained
                        nc.scalar.activation(out=yr[:, 384:768], in_=ps, func=ACT_COPY)
                    else:
                        nc.vector.tensor_copy(out=yr[:, 384:768], in_=ps)
                    nc.sync.dma_start(out=y_d.ap()[bass.ts(qt, 128), :], in_=yr)

            def drain_po(h, qh, po, qt_base):
                # fused normalize: o = O[:, 0:64] / O[:, 64] at psum drain
                pair, off = h // 2, (h % 2) * HD
                rcp = rcp_pool.tile([128, 4], f32, name="rcp", tag="rcp")
                nc.vector.reciprocal(out=rcp, in_=po[:, :, HD])
                for j in range(4):
                    qt = qh * 8 + qt_base + j
                    nc.vector.tensor_scalar_mul(
                        out=o_sb[:, qt, pair, off:off + HD],
                        in0=po[:, j, 0:HD], scalar1=rcp[:, j:j + 1],
                    )

            spill = []   # closures: previous slab's trailing O-matmuls + drains

            def attend(h, qh, fillers):
                nonlocal spill
                hp, off = h // 2, (h % 2) * HD
                qTp, kTp = qk_tiles[hp]
                pts = pt_pool.tile([128, NKT, 1024], bf16, name="pts", tag="pt")
                po_t = [None, None]
                myspill = []

                def own_o(kt):
                    for qt in range(8):
                        po = po_t[qt // 4]
                        # start=True zeroes the whole 2KB psum bank, so only
                        # the first column of each po bank may assert it
                        nc.tensor.matmul(
                            po[:, qt % 4, :],
                            pts[:, kt, bass.ts(qt, 128)],
                            v_sb[:, kt, h, :],
                            start=(kt == 0 and qt % 4 == 0),
                            stop=(kt == NKT - 1),
                            skip_group_check=True,
                        )

                fi = 0
                # fillers may read tiles written by the previous slab's
                # drains (spill[2..3], emitted at j=2,3) — dependency
                # tracking is emission-ordered, so fillers wait until j=4
                fstart = 4 if spill else 0

                def emit_fillers(j):
                    nonlocal fi
                    if j < fstart:
                        return
                    span = NKT - fstart
                    upto = min(
                        (len(fillers) * (j - fstart + 1) + span - 1) // span,
                        len(fillers),
                    )
                    while fi < upto:
                        fillers[fi]()
                        fi += 1

                for j in range(NKT):
                    pss = pss_pool.tile([128, 1024], f32, name="pss", tag="pss")
                    for c2 in range(2):
                        nc.tensor.matmul(
                            pss[:, bass.ts(c2, 512)],
                            kTp[off:off + HD, bass.ts(j, 128)],
                            qTp[off:off + HD, bass.ds(qh * 1024 + c2 * 512, 512)],
                            start=True, stop=True,
                        )
                    nc.scalar.activation(out=pts[:, j, :], in_=pss, func=ACT_EXP)
                    if j < len(spill):
                        spill[j]()
                    if j == LAG:
                        po_t[0] = po_pool.tile([128, 4, HD + 1], f32, name="po0", tag="po")
                        po_t[1] = po_pool.tile([128, 4, HD + 1], f32, name="po1", tag="po")
                    if j >= LAG:
                        own_o(j - LAG)
                    emit_fillers(j)

                # trailing O-matmuls packed two per iteration so the psum
                # drains land at j=2,3 of the next slab — two iterations
                # before its own O-matmuls reallocate the po slots at j=4
                myspill.append(lambda: (own_o(NKT - 4), own_o(NKT - 3)))
                myspill.append(lambda: (own_o(NKT - 2), own_o(NKT - 1)))
                myspill.append(lambda: drain_po(h, qh, po_t[0], 0))
                myspill.append(lambda: drain_po(h, qh, po_t[1], 4))
                spill = myspill

            # ---- static filler schedule -------------------------------
            mk_pair(0)
            F = {s: [] for s in range(1, 13)}
            F[1] = (
                [lambda kt=kt: v_emit(kt) for kt in range(4)]
                + [lambda: qkv_group(0, 1, "k")]
                + [lambda kt=kt: v_emit(kt) for kt in range(4, 8)]
                + [lambda: qkv_group(0, 2, "k")]
                + [lambda kt=kt: v_emit(kt) for kt in range(8, 12)]
                + [lambda: qkv_group(0, 3, "k")]
                + [lambda kt=kt: v_emit(kt) for kt in range(12, 16)]
            )
            F[2] = [
                lambda: qkv_group(0, 2, "q"),
                lambda: qkv_group(0, 3, "q"),
                lambda: mk_pair(1),
                lambda: qkv_group(1, 0, "k"),
                lambda: qkv_group(1, 0, "q"),
            ]
            F[3] = (
                [lambda: qkv_group(1, 1, "k"),
                 lambda: qkv_group(1, 1, "q"),
                 lambda: qkv_group(1, 2, "k")]
                + [lambda qt=qt: transpose_qt(0, qt) for qt in range(8)]
            )
            F[4] = [
                lambda: qkv_group(1, 2, "q"),
                lambda: qkv_group(1, 3, "k"),
                lambda: qkv_group(1, 3, "q"),
            ]
            F[5] = (
                [lambda: mk_pair(2),
                 lambda: qkv_group(2, 0, "k"),
                 lambda: qkv_group(2, 0, "q")]
                + [lambda qt=qt: transpose_qt(0, qt) for qt in range(8, 16)]
            )
            F[6] = [
                lambda: qkv_group(2, 1, "k"),
                lambda: qkv_group(2, 1, "q"),
            ]
            F[7] = (
                [lambda: qkv_group(2, 2, "k"),
                 lambda: qkv_group(2, 2, "q")]
                + [lambda qt=qt: transpose_qt(1, qt) for qt in range(8)]
            )
            F[8] = [
                lambda: qkv_group(2, 3, "k"),
                lambda: qkv_group(2, 3, "q"),
            ]
            F[9] = [lambda qt=qt: transpose_qt(1, qt) for qt in range(8, 16)]
            F[10] = []
            F[11] = (
                [lambda qt=qt: transpose_qt(2, qt) for qt in range(8)]
                + [lambda qt=qt, nh=nh: proj_group(qt, nh)
                   for qt in range(4) for nh in range(2)]
            )
            F[12] = [lambda qt=qt, nh=nh: proj_group(qt, nh)
                     for qt in range(4, 8) for nh in range(2)]

            # ---- startup: pair-0 chunks needed by the first slab, with
            # the three accumulations interleaved by dt so each matmul runs
            # as its xt tile lands (q-ch1 borrows a po slot; aux has 2)
            qTp0, kTp0 = qk_tiles[0]
            ps_k0 = aux_pool.tile([128, 512], f32, name="ps_k0", tag="aux")
            ps_q01 = pss_pool.tile([128, 1024], f32, name="ps_q01", tag="pss")
            for dt in range(NDT):
                nc.tensor.matmul(ps_k0, wk_sb[:, dt, 0:128], xt_dts[dt][:, 0:512],
                                 start=(dt == 0), stop=(dt == NDT - 1))
                nc.tensor.matmul(ps_q01[:, 0:512], wq_sb[:, dt, 0:128],
                                 xt_dts[dt][:, 0:512],
                                 start=(dt == 0), stop=(dt == NDT - 1))
                nc.tensor.matmul(ps_q01[:, 512:1024], wq_sb[:, dt, 0:128],
                                 xt_dts[dt][:, 512:1024],
                                 start=(dt == 0), stop=(dt == NDT - 1))
                if dt < NDT - 1:
                    warm(3)
            nc.vector.tensor_scalar_add(out=kTp0[:, 0:512], in0=ps_k0,
                                        scalar1=bk_sb[:, 0:1])
            nc.vector.tensor_scalar_add(out=qTp0[:, 0:1024], in0=ps_q01,
                                        scalar1=bqs_sb[:, 0:1])

            for s, (h, qh) in enumerate(SLABS, start=1):
                attend(h, qh, F[s])

            # ---- tail -------------------------------------------------
            for fn in spill:
                fn()
            for qt in range(8, 16):
                transpose_qt(2, qt, tail=True)
            for qt in range(8, 16):
                for nh in range(2):
                    proj_group(qt, nh, tail=True)

    nc.finalize()
    return nc


def _shard_inputs(x, w_qkv, b_qkv, w_proj):
    import ml_dtypes

    bf16 = ml_dtypes.bfloat16
    in_maps = []
    for c in range(NCORES):
        b, g = c // 2, c % 2
        sl = slice(DL * g, DL * g + DL)
        in_maps.append({
            "xt": np.ascontiguousarray(x[b].T).astype(bf16),
            "wq": np.ascontiguousarray(w_qkv[:, sl] * SCALE).astype(bf16),
            "wk": np.ascontiguousarray(w_qkv[:, EMBED:][:, sl]).astype(bf16),
            "wv": np.ascontiguousarray(w_qkv[:, 2 * EMBED:][:, sl]).astype(bf16),
            "bqs": np.ascontiguousarray(b_qkv[sl] * SCALE),
            "bk": np.ascontiguousarray(b_qkv[EMBED:][sl]),
            "wp": np.ascontiguousarray(w_proj[sl, :]).astype(bf16),
        })
    return in_maps


def kernel(x, w_qkv, b_qkv, w_proj, b_proj, _profile=False, _repeat=1):
    from concourse.bass_utils import run_bass_kernel_spmd

    x = np.asarray(x, dtype=np.float32)
    w_qkv = np.asarray(w_qkv, dtype=np.float32)
    b_qkv = np.asarray(b_qkv, dtype=np.float32)
    w_proj = np.asarray(w_proj, dtype=np.float32)
    b_proj = np.asarray(b_proj, dtype=np.float32)

    if _repeat not in _prog_cache:
        _prog_cache[_repeat] = _build_program(_repeat)
    nc = _prog_cache[_repeat]

    in_maps = _shard_inputs(x, w_qkv, b_qkv, w_proj)
    res = run_bass_kernel_spmd(
        nc, in_maps, list(range(NCORES)), trace=_profile,
    )

    # host-side gather: sum the two head-group partials per batch and add
    # the bias row (v-bias folded through w_proj, plus b_proj itself)
    bias_row = b_qkv[2 * EMBED:] @ w_proj + b_proj
    y = np.empty((B, T, EMBED), dtype=np.float32)
    for b in range(B):
        y[b] = (res.results[2 * b]["y"].astype(np.float32)
                + res.results[2 * b + 1]["y"].astype(np.float32) + bias_row)
    if _profile:
        return y, res
    return y


# revision 39
# speedup vs baseline: 1.1815x; 1.0048x over previous
"""Multi-head attention (B=4, T=2048, D=768, H=12) on 8 NeuronCores.

Sharding: core c handles batch b = c//2 and head-group g = c%2 (heads
6g..6g+5).  Each core computes its 6 heads' attention and a partial
output projection (contraction over its 384 local dims of w_proj); the
host sums the two partials per batch and adds the bias row.

Device formulation (bf16 matmul operands everywhere, fp32 psum):
  qT = Wq'.T @ xT  [384, 2048]   (Wq' pre-scaled by 1/sqrt(hd) on host)
  kT = Wk.T @ xT   [384, 2048]
  v  = x @ Wv      [2048, 384]   per kpos tile (65th column = 1.0)
  S^T[kt] = kT_h.T @ qT_h   [128 kpos, 1024 q]  per head, q-slab halves
  P^T = exp(S^T)   (ACT; scores max ~8 so no max subtraction)
  O[q, 65] += P^T[kt].T @ v'[kt]   <- flipped: output partitions = 128 q
      (col 64 accumulates the softmax denominators via the ones column)
  o = O[:, 0:64] * rcp(O[:, 64])   fused normalize in the psum drain
  oT via PE transpose (identity stationary), y = oT.T @ Wp per q tile.

The flip halves the P@V matmul cost vs the [65, 512]-output orientation
(the timeline cost model charges out-free-size cycles per matmul, so
output partition utilization is what matters).  Schedule: ACT (exp)
paces the attention inner loop at ~1.04us per [128,1024] tile; S, O,
QKV-projection, V, transposes and the output projection are spread
across the 12 (head, q-slab) sweeps to keep PE under that pace.
O-matmuls trail their exp by 4 kt iterations so psum-slot drains (DVE)
never stall the PE queue head.
"""

import numpy as np

EMBED = 768
HEADS = 12
HD = 64
SCALE = HD ** -0.5
B, T = 4, 2048
NCORES = 8
HPC = 6            # heads per core
DL = HPC * HD      # 384 local model dims per core

_prog_cache = {}


def _build_program(repeat=1):
    import concourse.bass as bass
    import concourse.mybir as mybir
    import concourse.tile as tile
    from concourse import bacc

    f32 = mybir.dt.float32
    f32r = mybir.dt.float32r
    bf16 = mybir.dt.bfloat16
    ACT_EXP = mybir.ActivationFunctionType.Exp
    ACT_COPY = mybir.ActivationFunctionType.Copy

    nc = bacc.Bacc()

    xt_d = nc.dram_tensor("xt", [EMBED, T], bf16, kind="ExternalInput")
    wq_d = nc.dram_tensor("wq", [EMBED, DL], bf16, kind="ExternalInput")
    wk_d = nc.dram_tensor("wk", [EMBED, DL], bf16, kind="ExternalInput")
    wv_d = nc.dram_tensor("wv", [EMBED, DL], bf16, kind="ExternalInput")
    bqs_d = nc.dram_tensor("bqs", [DL], f32, kind="ExternalInput")
    bk_d = nc.dram_tensor("bk", [DL], f32, kind="ExternalInput")
    wp_d = nc.dram_tensor("wp", [DL, EMBED], bf16, kind="ExternalInput")
    id_d = nc.dram_tensor("ident", [128, 128], f32r, kind="ExternalInput")
    y_d = nc.dram_tensor("y", [T, EMBED], bf16, kind="ExternalOutput")

    NDT = EMBED // 128   # 6 contraction tiles over embed dim
    NKT = T // 128       # 16 key-position tiles
    NQT = T // 128       # 16 query row tiles
    LAG = 4              # O-matmul lag (in kt iterations) behind exp

    # (head, q-slab) sweep order: q-major within each head pair so a
    # pair's q-half completes as early as possible (feeds transposes).
    SLABS = [(0, 0), (1, 0), (0, 1), (1, 1),
             (2, 0), (3, 0), (2, 1), (3, 1),
             (4, 0), (5, 0), (4, 1), (5, 1)]

    with tile.TileContext(nc) as tc:
      for _rep in range(repeat):
        with tc.tile_pool(name="pers", bufs=1) as pers, \
             tc.tile_pool(name="qk", bufs=2) as qk_pool, \
             tc.tile_pool(name="pt", bufs=2) as pt_pool, \
             tc.tile_pool(name="rcp", bufs=4) as rcp_pool, \
             tc.tile_pool(name="yr", bufs=3) as yr_pool, \
             tc.tile_pool(name="pss", bufs=2, space="PSUM") as pss_pool, \
             tc.tile_pool(name="po", bufs=2, space="PSUM") as po_pool, \
             tc.tile_pool(name="aux", bufs=2, space="PSUM") as aux_pool:

            xt_dts = [pers.tile([128, T], bf16, name=f"xt{dt}_sb")
                      for dt in range(NDT)]
            wq_sb = pers.tile([128, NDT, DL], bf16, name="wq_sb")
            wk_sb = pers.tile([128, NDT, DL], bf16, name="wk_sb")
            wv_sb = pers.tile([128, NDT, DL], bf16, name="wv_sb")
            wp_sb = pers.tile([128, 3, EMBED], bf16, name="wp_sb")
            v_sb = pers.tile([128, NKT, HPC, HD + 1], bf16, name="v_sb")
            bqs_sb = pers.tile([128, 3], f32, name="bqs_sb")
            bk_sb = pers.tile([128, 3], f32, name="bk_sb")
            o_sb = pers.tile([128, NQT, 3, 128], bf16, name="o_sb")
            # pair-2 qh1 stages in f32r so the tail can transpose on the PE
            # (no serial HWDGE descriptor-gens on the critical tail path)
            o2q1_sb = pers.tile([128, 8, 128], f32r, name="o2q1_sb")
            id_sb = pers.tile([128, 128], f32r, name="id_sb")
            oT_sb = pers.tile([128, 3, T], bf16, name="oT_sb")
            warm_sb = pers.tile([128, 256], bf16, name="warm_sb")

            # ones column of v' (softmax denominator accumulator) — only
            # the 65th columns; emitted first so it doesn't sit behind the
            # DMA descriptor generation on the Pool queue
            nc.gpsimd.memset(v_sb[:, :, :, HD:HD + 1], 1.0)
            nc.vector.memset(warm_sb, 0.0)

            # input DMAs: first-slab critical path is wk/wq + all of xt
            # (full embed contraction).  The DMA copies serialize on one
            # resource, so everything not needed before the first S goes
            # after xt.  Per-dt xt tiles give each transfer its own
            # completion sem (DMA write deps are tile x queue granular).
            nc.gpsimd.dma_start(out=wk_sb, in_=wk_d.ap().rearrange("(n p) m -> p n m", p=128))
            nc.gpsimd.dma_start(out=wq_sb, in_=wq_d.ap().rearrange("(n p) m -> p n m", p=128))
            for dt in range(3):
                nc.sync.dma_start(out=xt_dts[dt], in_=xt_d.ap()[bass.ts(dt, 128), :])
            for dt in range(3, NDT):
                nc.gpsimd.dma_start(out=xt_dts[dt], in_=xt_d.ap()[bass.ts(dt, 128), :])
            nc.gpsimd.dma_start(out=wv_sb, in_=wv_d.ap().rearrange("(n p) m -> p n m", p=128))
            nc.gpsimd.dma_start(out=wp_sb, in_=wp_d.ap().rearrange("(n p) m -> p n m", p=128))
            nc.gpsimd.dma_start(out=id_sb, in_=id_d.ap())
            nc.sync.dma_start(out=bqs_sb, in_=bqs_d.ap().rearrange("(n p) -> p n", p=128))
            nc.sync.dma_start(out=bk_sb, in_=bk_d.ap().rearrange("(n p) -> p n", p=128))

            def warm(n):
                # warm-up matmuls ride the po slots (idle until the first
                # slab's O accumulation; pss holds ps_q01 through startup)
                for _w in range(n):
                    psw = po_pool.tile([128, 256], f32, name="psw", tag="po")
                    nc.tensor.matmul(psw, warm_sb[0:2, 0:128], warm_sb[0:2, :],
                                     start=True, stop=True)

            warm(6)

            qk_tiles = {}
            yr_tiles = {}

            def mk_pair(hp):
                qk_tiles[hp] = (
                    qk_pool.tile([128, T], bf16, name="qTp", tag="qT"),
                    qk_pool.tile([128, T], bf16, name="kTp", tag="kT"),
                )

            def qkv_group(hp, ch, which):
                csl = bass.ts(ch, 512)
                qTp, kTp = qk_tiles[hp]
                dst, wsb, bias = (
                    (qTp, wq_sb, bqs_sb) if which == "q" else (kTp, wk_sb, bk_sb)
                )
                ps = aux_pool.tile([128, 512], f32, name="psqk", tag="aux")
                for dt in range(NDT):
                    nc.tensor.matmul(
                        ps, wsb[:, dt, bass.ts(hp, 128)], xt_dts[dt][:, csl],
                        start=(dt == 0), stop=(dt == NDT - 1),
                    )
                nc.vector.tensor_scalar_add(
                    out=dst[:, csl], in0=ps, scalar1=bias[:, hp:hp + 1],
                )

            def v_emit(kt):
                ps = aux_pool.tile([128, DL], f32, name="psv", tag="aux")
                for dt in range(NDT):
                    nc.tensor.matmul(
                        ps, xt_dts[dt][:, bass.ts(kt, 128)], wv_sb[:, dt, :],
                        start=(dt == 0), stop=(dt == NDT - 1),
                    )
                # GPSIMD cannot touch PSUM; DVE is nearly idle during the
                # V-emission slab (ACT copies here would stall its in-order
                # queue ahead of the exps)
                nc.vector.tensor_copy(
                    out=v_sb[:, kt, :, 0:HD],
                    in_=ps.rearrange("p (h d) -> p h d", h=HPC),
                )

            def transpose_qt(pair, qt, tail=False):
                # 2-byte dtypes transpose on the DMA xbar (PE transpose into
                # psum is 4-byte-cell granular and corrupts bf16).  Tail
                # transposes issue from the otherwise-idle ACT queue so their
                # descriptor generation doesn't serialize behind the y DMAs.
                eng = nc.scalar if tail else nc.sync
                eng.dma_start_transpose(
                    out=oT_sb[:, pair, bass.ts(qt, 128)],
                    in_=o_sb[:, qt, pair, :],
                )

            def proj_group(qt, nh, tail=False):
                ps = aux_pool.tile([128, 384], f32, name="psy", tag="aux")
                for dtp in range(3):
                    nc.tensor.matmul(
                        ps, oT_sb[:, dtp, bass.ts(qt, 128)],
                        wp_sb[:, dtp, bass.ts(nh, 384)],
                        start=(dtp == 0), stop=(dtp == 2),
                    )
                if nh == 0:
                    yr = yr_pool.tile([128, EMBED], bf16, name="yr", tag="yr")
                    yr_tiles[qt] = yr
                    nc.vector.tensor_copy(out=yr[:, 0:384], in_=ps)
                else:
                    yr = yr_tiles.pop(qt)
                    if tail:  # ACT is idle once attention has drained
                        nc.scalar.activation(out=yr[:, 384:768], in_=ps, func=ACT_COPY)
                    else:
                        nc.vector.tensor_copy(out=yr[:, 384:768], in_=ps)
                    nc.sync.dma_start(out=y_d.ap()[bass.ts(qt, 128), :], in_=yr)

            def drain_po(h, qh, po, qt_base):
                # fused normalize: o = O[:, 0:64] / O[:, 64] at psum drain
                pair, off = h // 2, (h % 2) * HD
                rcp = rcp_pool.tile([128, 4], f32, name="rcp", tag="rcp")
                nc.vector.reciprocal(out=rcp, in_=po[:, :, HD])
                for j in range(4):
                    qt = qh * 8 + qt_base + j
                    if pair == 2 and qh == 1:
                        dst = o2q1_sb[:, qt - 8, off:off + HD]
                    else:
                        dst = o_sb[:, qt, pair, off:off + HD]
                    with nc.allow_low_precision(reason="f32r staging"):
                        nc.vector.tensor_scalar_mul(
                            out=dst, in0=po[:, j, 0:HD], scalar1=rcp[:, j:j + 1],
                        )

            spill = []   # closures: previous slab's trailing O-matmuls + drains

            def attend(h, qh, fillers):
                nonlocal spill
                hp, off = h // 2, (h % 2) * HD
                qTp, kTp = qk_tiles[hp]
                pts = pt_pool.tile([128, NKT, 1024], bf16, name="pts", tag="pt")
                po_t = [None, None]
                myspill = []

                def own_o(kt):
                    for qt in range(8):
                        po = po_t[qt // 4]
                        # start=True zeroes the whole 2KB psum bank, so only
                        # the first column of each po bank may assert it
                        nc.tensor.matmul(
                            po[:, qt % 4, :],
                            pts[:, kt, bass.ts(qt, 128)],
                            v_sb[:, kt, h, :],
                            start=(kt == 0 and qt % 4 == 0),
                            stop=(kt == NKT - 1),
                            skip_group_check=True,
                        )

                fi = 0
                # fillers may read tiles written by the previous slab's
                # drains (spill[2..3], emitted at j=2,3) — dependency
                # tracking is emission-ordered, so fillers wait until j=4
                fstart = 4 if spill else 0

                def emit_fillers(j):
                    nonlocal fi
                    if j < fstart:
                        return
                    span = NKT - fstart
                    upto = min(
                        (len(fillers) * (j - fstart + 1) + span - 1) // span,
                        len(fillers),
                    )
                    while fi < upto:
                        fillers[fi]()
                        fi += 1

                for j in range(NKT):
                    pss = pss_pool.tile([128, 1024], f32, name="pss", tag="pss")
                    for c2 in range(2):
                        nc.tensor.matmul(
                            pss[:, bass.ts(c2, 512)],
                            kTp[off:off + HD, bass.ts(j, 128)],
                            qTp[off:off + HD, bass.ds(qh * 1024 + c2 * 512, 512)],
                            start=True, stop=True,
                        )
                    nc.scalar.activation(out=pts[:, j, :], in_=pss, func=ACT_EXP)
                    if j < len(spill):
                        spill[j]()
                    if j == LAG:
                        po_t[0] = po_pool.tile([128, 4, HD + 1], f32, name="po0", tag="po")
                        po_t[1] = po_pool.tile([128, 4, HD + 1], f32, name="po1", tag="po")
                    if j >= LAG:
                        own_o(j - LAG)
                    emit_fillers(j)

                # trailing O-matmuls packed two per iteration so the psum
                # drains land at j=2,3 of the next slab — two iterations
                # before its own O-matmuls reallocate the po slots at j=4
                myspill.append(lambda: (own_o(NKT - 4), own_o(NKT - 3)))
                myspill.append(lambda: (own_o(NKT - 2), own_o(NKT - 1)))
                myspill.append(lambda: drain_po(h, qh, po_t[0], 0))
                myspill.append(lambda: drain_po(h, qh, po_t[1], 4))
                spill = myspill

            # ---- static filler schedule -------------------------------
            mk_pair(0)
            F = {s: [] for s in range(1, 13)}
            F[1] = (
                [lambda kt=kt: v_emit(kt) for kt in range(4)]
                + [lambda: qkv_group(0, 1, "k")]
                + [lambda kt=kt: v_emit(kt) for kt in range(4, 8)]
                + [lambda: qkv_group(0, 2, "k")]
                + [lambda kt=kt: v_emit(kt) for kt in range(8, 12)]
                + [lambda: qkv_group(0, 3, "k")]
                + [lambda kt=kt: v_emit(kt) for kt in range(12, 16)]
            )
            F[2] = [
                lambda: qkv_group(0, 2, "q"),
                lambda: qkv_group(0, 3, "q"),
                lambda: mk_pair(1),
                lambda: qkv_group(1, 0, "k"),
                lambda: qkv_group(1, 0, "q"),
            ]
            F[3] = (
                [lambda: qkv_group(1, 1, "k"),
                 lambda: qkv_group(1, 1, "q"),
                 lambda: qkv_group(1, 2, "k")]
                + [lambda qt=qt: transpose_qt(0, qt) for qt in range(8)]
            )
            F[4] = [
                lambda: qkv_group(1, 2, "q"),
                lambda: qkv_group(1, 3, "k"),
                lambda: qkv_group(1, 3, "q"),
            ]
            F[5] = (
                [lambda: mk_pair(2),
                 lambda: qkv_group(2, 0, "k"),
                 lambda: qkv_group(2, 0, "q")]
                + [lambda qt=qt: transpose_qt(0, qt) for qt in range(8, 16)]
            )
            F[6] = [
                lambda: qkv_group(2, 1, "k"),
                lambda: qkv_group(2, 1, "q"),
            ]
            F[7] = (
                [lambda: qkv_group(2, 2, "k"),
                 lambda: qkv_group(2, 2, "q")]
                + [lambda qt=qt: transpose_qt(1, qt) for qt in range(8)]
            )
            F[8] = [
                lambda: qkv_group(2, 3, "k"),
                lambda: qkv_group(2, 3, "q"),
            ]
            F[9] = [lambda qt=qt: transpose_qt(1, qt) for qt in range(8, 16)]
            F[10] = []
            F[11] = (
                [lambda qt=qt: transpose_qt(2, qt) for qt in range(8)]
                + [lambda qt=qt, nh=nh: proj_group(qt, nh)
                   for qt in range(4) for nh in range(2)]
            )
            F[12] = [lambda qt=qt, nh=nh: proj_group(qt, nh)
                     for qt in range(4, 8) for nh in range(2)]

            # ---- startup: pair-0 chunks needed by the first slab, with
            # the three accumulations interleaved by dt so each matmul runs
            # as its xt tile lands (q-ch1 borrows a po slot; aux has 2)
            qTp0, kTp0 = qk_tiles[0]
            ps_k0 = aux_pool.tile([128, 512], f32, name="ps_k0", tag="aux")
            ps_q01 = pss_pool.tile([128, 1024], f32, name="ps_q01", tag="pss")
            for dt in range(NDT):
                nc.tensor.matmul(ps_k0, wk_sb[:, dt, 0:128], xt_dts[dt][:, 0:512],
                                 start=(dt == 0), stop=(dt == NDT - 1))
                nc.tensor.matmul(ps_q01[:, 0:512], wq_sb[:, dt, 0:128],
                                 xt_dts[dt][:, 0:512],
                                 start=(dt == 0), stop=(dt == NDT - 1))
                nc.tensor.matmul(ps_q01[:, 512:1024], wq_sb[:, dt, 0:128],
                                 xt_dts[dt][:, 512:1024],
                                 start=(dt == 0), stop=(dt == NDT - 1))
                if dt < NDT - 1:
                    warm(3)
            nc.vector.tensor_scalar_add(out=kTp0[:, 0:512], in0=ps_k0,
                                        scalar1=bk_sb[:, 0:1])
            nc.vector.tensor_scalar_add(out=qTp0[:, 0:1024], in0=ps_q01,
                                        scalar1=bqs_sb[:, 0:1])

            for s, (h, qh) in enumerate(SLABS, start=1):
                attend(h, qh, F[s])

            # ---- tail -------------------------------------------------
            for fn in spill:
                fn()
            for qt in range(8, 16):
                # PE transpose (f32r) + DVE copy: no HWDGE gen on the tail
                pst = aux_pool.tile([128, 128], f32r, name="pst", tag="aux")
                nc.tensor.matmul(pst, o2q1_sb[:, qt - 8, :], id_sb,
                                 is_transpose=True)
                nc.vector.tensor_copy(out=oT_sb[:, 2, bass.ts(qt, 128)], in_=pst)
                for nh in range(2):
                    proj_group(qt, nh, tail=True)

    nc.finalize()
    return nc


def _shard_inputs(x, w_qkv, b_qkv, w_proj):
    import ml_dtypes

    bf16 = ml_dtypes.bfloat16
    in_maps = []
    for c in range(NCORES):
        b, g = c // 2, c % 2
        sl = slice(DL * g, DL * g + DL)
        in_maps.append({
            "xt": np.ascontiguousarray(x[b].T).astype(bf16),
            "wq": np.ascontiguousarray(w_qkv[:, sl] * SCALE).astype(bf16),
            "wk": np.ascontiguousarray(w_qkv[:, EMBED:][:, sl]).astype(bf16),
            "wv": np.ascontiguousarray(w_qkv[:, 2 * EMBED:][:, sl]).astype(bf16),
            "bqs": np.ascontiguousarray(b_qkv[sl] * SCALE),
            "bk": np.ascontiguousarray(b_qkv[EMBED:][sl]),
            "wp": np.ascontiguousarray(w_proj[sl, :]).astype(bf16),
            "ident": np.eye(128, dtype=np.float32),
        })
    return in_maps


def kernel(x, w_qkv, b_qkv, w_proj, b_proj, _profile=False, _repeat=1):
    from concourse.bass_utils import run_bass_kernel_spmd

    x = np.asarray(x, dtype=np.float32)
    w_qkv = np.asarray(w_qkv, dtype=np.float32)
    b_qkv = np.asarray(b_qkv, dtype=np.float32)
    w_proj = np.asarray(w_proj, dtype=np.float32)
    b_proj = np.asarray(b_proj, dtype=np.float32)

    if _repeat not in _prog_cache:
        _prog_cache[_repeat] = _build_program(_repeat)
    nc = _prog_cache[_repeat]

    in_maps = _shard_inputs(x, w_qkv, b_qkv, w_proj)
    res = run_bass_kernel_spmd(
        nc, in_maps, list(range(NCORES)), trace=_profile,
    )

    # host-side gather: sum the two head-group partials per batch and add
    # the bias row (v-bias folded through w_proj, plus b_proj itself)
    bias_row = b_qkv[2 * EMBED:] @ w_proj + b_proj
    y = np.empty((B, T, EMBED), dtype=np.float32)
    for b in range(B):
        y[b] = (res.results[2 * b]["y"].astype(np.float32)
                + res.results[2 * b + 1]["y"].astype(np.float32) + bias_row)
    if _profile:
        return y, res
    return y


# revision 41
# speedup vs baseline: 1.1897x; 1.0069x over previous
"""Multi-head attention (B=4, T=2048, D=768, H=12) on 8 NeuronCores.

Sharding: core c handles batch b = c//2 and head-group g = c%2 (heads
6g..6g+5).  Each core computes its 6 heads' attention and a partial
output projection (contraction over its 384 local dims of w_proj); the
host sums the two partials per batch and adds the bias row.

Device formulation (bf16 matmul operands everywhere, fp32 psum):
  qT = Wq'.T @ xT  [384, 2048]   (Wq' pre-scaled by 1/sqrt(hd) on host)
  kT = Wk.T @ xT   [384, 2048]
  v  = x @ Wv      [2048, 384]   per kpos tile (65th column = 1.0)
  S^T[kt] = kT_h.T @ qT_h   [128 kpos, 1024 q]  per head, q-slab halves
  P^T = exp(S^T)   (ACT; scores max ~8 so no max subtraction)
  O[q, 65] += P^T[kt].T @ v'[kt]   <- flipped: output partitions = 128 q
      (col 64 accumulates the softmax denominators via the ones column)
  o = O[:, 0:64] * rcp(O[:, 64])   fused normalize in the psum drain
  oT via PE transpose (identity stationary), y = oT.T @ Wp per q tile.

The flip halves the P@V matmul cost vs the [65, 512]-output orientation
(the timeline cost model charges out-free-size cycles per matmul, so
output partition utilization is what matters).  Schedule: ACT (exp)
paces the attention inner loop at ~1.04us per [128,1024] tile; S, O,
QKV-projection, V, transposes and the output projection are spread
across the 12 (head, q-slab) sweeps to keep PE under that pace.
O-matmuls trail their exp by 4 kt iterations so psum-slot drains (DVE)
never stall the PE queue head.
"""

import numpy as np

EMBED = 768
HEADS = 12
HD = 64
SCALE = HD ** -0.5
B, T = 4, 2048
NCORES = 8
HPC = 6            # heads per core
DL = HPC * HD      # 384 local model dims per core

_prog_cache = {}


def _build_program(repeat=1):
    import concourse.bass as bass
    import concourse.mybir as mybir
    import concourse.tile as tile
    from concourse import bacc

    f32 = mybir.dt.float32
    f32r = mybir.dt.float32r
    bf16 = mybir.dt.bfloat16
    ACT_EXP = mybir.ActivationFunctionType.Exp
    ACT_COPY = mybir.ActivationFunctionType.Copy

    nc = bacc.Bacc()

    xt_d = nc.dram_tensor("xt", [EMBED, T], bf16, kind="ExternalInput")
    wq_d = nc.dram_tensor("wq", [EMBED, DL], bf16, kind="ExternalInput")
    wk_d = nc.dram_tensor("wk", [EMBED, DL], bf16, kind="ExternalInput")
    wv_d = nc.dram_tensor("wv", [EMBED, DL], bf16, kind="ExternalInput")
    bqs_d = nc.dram_tensor("bqs", [DL], f32, kind="ExternalInput")
    bk_d = nc.dram_tensor("bk", [DL], f32, kind="ExternalInput")
    wp_d = nc.dram_tensor("wp", [DL, EMBED], bf16, kind="ExternalInput")
    id_d = nc.dram_tensor("ident", [128, 128], f32r, kind="ExternalInput")
    y_d = nc.dram_tensor("y", [T, EMBED], bf16, kind="ExternalOutput")

    NDT = EMBED // 128   # 6 contraction tiles over embed dim
    NKT = T // 128       # 16 key-position tiles
    NQT = T // 128       # 16 query row tiles
    LAG = 4              # O-matmul lag (in kt iterations) behind exp

    # (head, q-slab) sweep order: q-major within each head pair so a
    # pair's q-half completes as early as possible (feeds transposes).
    SLABS = [(0, 0), (1, 0), (0, 1), (1, 1),
             (2, 0), (3, 0), (2, 1), (3, 1),
             (4, 0), (5, 0), (4, 1), (5, 1)]

    with tile.TileContext(nc) as tc:
      for _rep in range(repeat):
        with tc.tile_pool(name="pers", bufs=1) as pers, \
             tc.tile_pool(name="qk", bufs=2) as qk_pool, \
             tc.tile_pool(name="pt", bufs=2) as pt_pool, \
             tc.tile_pool(name="rcp", bufs=4) as rcp_pool, \
             tc.tile_pool(name="yr", bufs=3) as yr_pool, \
             tc.tile_pool(name="pss", bufs=2, space="PSUM") as pss_pool, \
             tc.tile_pool(name="po", bufs=2, space="PSUM") as po_pool, \
             tc.tile_pool(name="aux", bufs=2, space="PSUM") as aux_pool:

            xt_dts = [pers.tile([128, T], bf16, name=f"xt{dt}_sb")
                      for dt in range(NDT)]
            wq_sb = pers.tile([128, NDT, DL], bf16, name="wq_sb")
            wk_sb = pers.tile([128, NDT, DL], bf16, name="wk_sb")
            wv_sb = pers.tile([128, NDT, DL], bf16, name="wv_sb")
            wp_sb = pers.tile([128, 3, EMBED], bf16, name="wp_sb")
            v_sb = pers.tile([128, NKT, HPC, HD + 1], bf16, name="v_sb")
            bqs_sb = pers.tile([128, 3], f32, name="bqs_sb")
            bk_sb = pers.tile([128, 3], f32, name="bk_sb")
            o_sb = pers.tile([128, NQT, 3, 128], bf16, name="o_sb")
            # pair-2 qh1 stages in f32r so the tail can transpose on the PE
            # (no serial HWDGE descriptor-gens on the critical tail path)
            o2q1_sb = pers.tile([128, 8, 128], f32r, name="o2q1_sb")
            id_sb = pers.tile([128, 128], f32r, name="id_sb")
            oT_sb = pers.tile([128, 3, T], bf16, name="oT_sb")
            warm_sb = pers.tile([128, 256], bf16, name="warm_sb")

            # ones column of v' (softmax denominator accumulator) — only
            # the 65th columns; emitted first so it doesn't sit behind the
            # DMA descriptor generation on the Pool queue
            nc.gpsimd.memset(v_sb[:, :, :, HD:HD + 1], 1.0)
            nc.vector.memset(warm_sb, 0.0)

            # input DMAs: first-slab critical path is wk/wq + all of xt
            # (full embed contraction).  The DMA copies serialize on one
            # resource, so everything not needed before the first S goes
            # after xt.  Per-dt xt tiles give each transfer its own
            # completion sem (DMA write deps are tile x queue granular).
            nc.gpsimd.dma_start(out=wk_sb, in_=wk_d.ap().rearrange("(n p) m -> p n m", p=128))
            nc.gpsimd.dma_start(out=wq_sb, in_=wq_d.ap().rearrange("(n p) m -> p n m", p=128))
            for dt in range(3):
                nc.sync.dma_start(out=xt_dts[dt], in_=xt_d.ap()[bass.ts(dt, 128), :])
            for dt in range(3, NDT):
                nc.gpsimd.dma_start(out=xt_dts[dt], in_=xt_d.ap()[bass.ts(dt, 128), :])
            nc.gpsimd.dma_start(out=wv_sb, in_=wv_d.ap().rearrange("(n p) m -> p n m", p=128))
            nc.gpsimd.dma_start(out=wp_sb, in_=wp_d.ap().rearrange("(n p) m -> p n m", p=128))
            nc.gpsimd.dma_start(out=id_sb, in_=id_d.ap())
            nc.sync.dma_start(out=bqs_sb, in_=bqs_d.ap().rearrange("(n p) -> p n", p=128))
            nc.sync.dma_start(out=bk_sb, in_=bk_d.ap().rearrange("(n p) -> p n", p=128))

            def warm(n):
                # warm-up matmuls ride the po slots (idle until the first
                # slab's O accumulation; pss holds ps_q01 through startup)
                for _w in range(n):
                    psw = po_pool.tile([128, 256], f32, name="psw", tag="po")
                    nc.tensor.matmul(psw, warm_sb[0:2, 0:128], warm_sb[0:2, :],
                                     start=True, stop=True)

            warm(6)

            qk_tiles = {}
            yr_tiles = {}

            def mk_pair(hp):
                qk_tiles[hp] = (
                    qk_pool.tile([128, T], bf16, name="qTp", tag="qT"),
                    qk_pool.tile([128, T], bf16, name="kTp", tag="kT"),
                )

            def qkv_group(hp, ch, which):
                csl = bass.ts(ch, 512)
                qTp, kTp = qk_tiles[hp]
                dst, wsb, bias = (
                    (qTp, wq_sb, bqs_sb) if which == "q" else (kTp, wk_sb, bk_sb)
                )
                ps = aux_pool.tile([128, 512], f32, name="psqk", tag="aux")
                for dt in range(NDT):
                    nc.tensor.matmul(
                        ps, wsb[:, dt, bass.ts(hp, 128)], xt_dts[dt][:, csl],
                        start=(dt == 0), stop=(dt == NDT - 1),
                    )
                nc.vector.tensor_scalar_add(
                    out=dst[:, csl], in0=ps, scalar1=bias[:, hp:hp + 1],
                )

            def v_emit(kt):
                ps = aux_pool.tile([128, DL], f32, name="psv", tag="aux")
                for dt in range(NDT):
                    nc.tensor.matmul(
                        ps, xt_dts[dt][:, bass.ts(kt, 128)], wv_sb[:, dt, :],
                        start=(dt == 0), stop=(dt == NDT - 1),
                    )
                # GPSIMD cannot touch PSUM; DVE is nearly idle during the
                # V-emission slab (ACT copies here would stall its in-order
                # queue ahead of the exps)
                nc.vector.tensor_copy(
                    out=v_sb[:, kt, :, 0:HD],
                    in_=ps.rearrange("p (h d) -> p h d", h=HPC),
                )

            def transpose_qt(pair, qt, tail=False):
                # 2-byte dtypes transpose on the DMA xbar (PE transpose into
                # psum is 4-byte-cell granular and corrupts bf16).  Tail
                # transposes issue from the otherwise-idle ACT queue so their
                # descriptor generation doesn't serialize behind the y DMAs.
                eng = nc.scalar if tail else nc.sync
                eng.dma_start_transpose(
                    out=oT_sb[:, pair, bass.ts(qt, 128)],
                    in_=o_sb[:, qt, pair, :],
                )

            def proj_group(qt, nh, tail=False):
                ps = aux_pool.tile([128, 384], f32, name="psy", tag="aux")
                for dtp in range(3):
                    nc.tensor.matmul(
                        ps, oT_sb[:, dtp, bass.ts(qt, 128)],
                        wp_sb[:, dtp, bass.ts(nh, 384)],
                        start=(dtp == 0), stop=(dtp == 2),
                    )
                if nh == 0:
                    yr = yr_pool.tile([128, EMBED], bf16, name="yr", tag="yr")
                    yr_tiles[qt] = yr
                    nc.vector.tensor_copy(out=yr[:, 0:384], in_=ps)
                else:
                    yr = yr_tiles.pop(qt)
                    if tail:  # ACT is idle once attention has drained
                        nc.scalar.activation(out=yr[:, 384:768], in_=ps, func=ACT_COPY)
                    else:
                        nc.vector.tensor_copy(out=yr[:, 384:768], in_=ps)
                    nc.sync.dma_start(out=y_d.ap()[bass.ts(qt, 128), :], in_=yr)

            def drain_po(h, qh, po, qt_base):
                # fused normalize: o = O[:, 0:64] / O[:, 64] at psum drain
                pair, off = h // 2, (h % 2) * HD
                rcp = rcp_pool.tile([128, 4], f32, name="rcp", tag="rcp")
                nc.vector.reciprocal(out=rcp, in_=po[:, :, HD])
                for j in range(4):
                    qt = qh * 8 + qt_base + j
                    if pair == 2 and qh == 1:
                        dst = o2q1_sb[:, qt - 8, off:off + HD]
                    else:
                        dst = o_sb[:, qt, pair, off:off + HD]
                    with nc.allow_low_precision(reason="f32r staging"):
                        nc.vector.tensor_scalar_mul(
                            out=dst, in0=po[:, j, 0:HD], scalar1=rcp[:, j:j + 1],
                        )

            spill = []   # closures: previous slab's trailing O-matmuls + drains

            def attend(h, qh, fillers):
                nonlocal spill
                hp, off = h // 2, (h % 2) * HD
                qTp, kTp = qk_tiles[hp]
                pts = pt_pool.tile([128, NKT, 1024], bf16, name="pts", tag="pt")
                po_t = [None, None]
                myspill = []

                def own_o(kt):
                    for qt in range(8):
                        po = po_t[qt // 4]
                        # start=True zeroes the whole 2KB psum bank, so only
                        # the first column of each po bank may assert it
                        nc.tensor.matmul(
                            po[:, qt % 4, :],
                            pts[:, kt, bass.ts(qt, 128)],
                            v_sb[:, kt, h, :],
                            start=(kt == 0 and qt % 4 == 0),
                            stop=(kt == NKT - 1),
                            skip_group_check=True,
                        )

                fi = 0
                # fillers may read tiles written by the previous slab's
                # drains (spill[2..3], emitted at j=2,3) — dependency
                # tracking is emission-ordered, so fillers wait until j=4
                fstart = 4 if spill else 0

                def emit_fillers(j):
                    nonlocal fi
                    if j < fstart:
                        return
                    span = NKT - fstart
                    upto = min(
                        (len(fillers) * (j - fstart + 1) + span - 1) // span,
                        len(fillers),
                    )
                    while fi < upto:
                        fillers[fi]()
                        fi += 1

                for j in range(NKT):
                    pss = pss_pool.tile([128, 1024], f32, name="pss", tag="pss")
                    for c2 in range(2):
                        nc.tensor.matmul(
                            pss[:, bass.ts(c2, 512)],
                            kTp[off:off + HD, bass.ts(j, 128)],
                            qTp[off:off + HD, bass.ds(qh * 1024 + c2 * 512, 512)],
                            start=True, stop=True,
                        )
                    nc.scalar.activation(out=pts[:, j, :], in_=pss, func=ACT_EXP)
                    if j < len(spill):
                        spill[j]()
                    if j == LAG:
                        po_t[0] = po_pool.tile([128, 4, HD + 1], f32, name="po0", tag="po")
                        po_t[1] = po_pool.tile([128, 4, HD + 1], f32, name="po1", tag="po")
                    if j >= LAG:
                        own_o(j - LAG)
                    emit_fillers(j)

                # trailing O-matmuls packed two per iteration so the psum
                # drains land at j=2,3 of the next slab — two iterations
                # before its own O-matmuls reallocate the po slots at j=4
                myspill.append(lambda: (own_o(NKT - 4), own_o(NKT - 3)))
                myspill.append(lambda: (own_o(NKT - 2), own_o(NKT - 1)))
                myspill.append(lambda: drain_po(h, qh, po_t[0], 0))
                myspill.append(lambda: drain_po(h, qh, po_t[1], 4))
                spill = myspill

            # ---- static filler schedule -------------------------------
            mk_pair(0)
            F = {s: [] for s in range(1, 13)}
            F[1] = (
                [lambda kt=kt: v_emit(kt) for kt in range(4)]
                + [lambda: qkv_group(0, 1, "k")]
                + [lambda kt=kt: v_emit(kt) for kt in range(4, 8)]
                + [lambda: qkv_group(0, 2, "k")]
                + [lambda kt=kt: v_emit(kt) for kt in range(8, 12)]
                + [lambda: qkv_group(0, 3, "k")]
                + [lambda kt=kt: v_emit(kt) for kt in range(12, 16)]
            )
            F[2] = [
                lambda: qkv_group(0, 2, "q"),
                lambda: qkv_group(0, 3, "q"),
                lambda: mk_pair(1),
                lambda: qkv_group(1, 0, "k"),
                lambda: qkv_group(1, 0, "q"),
            ]
            F[3] = (
                [lambda: qkv_group(1, 1, "k"),
                 lambda: qkv_group(1, 1, "q"),
                 lambda: qkv_group(1, 2, "k")]
                + [lambda qt=qt: transpose_qt(0, qt) for qt in range(8)]
            )
            F[4] = [
                lambda: qkv_group(1, 2, "q"),
                lambda: qkv_group(1, 3, "k"),
                lambda: qkv_group(1, 3, "q"),
            ]
            F[5] = (
                [lambda: mk_pair(2),
                 lambda: qkv_group(2, 0, "k"),
                 lambda: qkv_group(2, 0, "q")]
                + [lambda qt=qt: transpose_qt(0, qt) for qt in range(8, 16)]
            )
            F[6] = [
                lambda: qkv_group(2, 1, "k"),
                lambda: qkv_group(2, 1, "q"),
            ]
            F[7] = (
                [lambda: qkv_group(2, 2, "k"),
                 lambda: qkv_group(2, 2, "q")]
                + [lambda qt=qt: transpose_qt(1, qt) for qt in range(8)]
            )
            F[8] = [
                lambda: qkv_group(2, 3, "k"),
                lambda: qkv_group(2, 3, "q"),
            ]
            F[9] = [lambda qt=qt: transpose_qt(1, qt) for qt in range(8, 16)]
            F[10] = []
            F[11] = (
                [lambda qt=qt: transpose_qt(2, qt) for qt in range(8)]
                + [lambda qt=qt, nh=nh: proj_group(qt, nh)
                   for qt in range(4) for nh in range(2)]
            )
            F[12] = [lambda qt=qt, nh=nh: proj_group(qt, nh)
                     for qt in range(4, 8) for nh in range(2)]

            # ---- startup: pair-0 chunks needed by the first slab, with
            # the three accumulations interleaved by dt so each matmul runs
            # as its xt tile lands (q-ch1 borrows a po slot; aux has 2)
            qTp0, kTp0 = qk_tiles[0]
            ps_k0 = aux_pool.tile([128, 512], f32, name="ps_k0", tag="aux")
            ps_q01 = pss_pool.tile([128, 1024], f32, name="ps_q01", tag="pss")
            for dt in range(NDT):
                nc.tensor.matmul(ps_k0, wk_sb[:, dt, 0:128], xt_dts[dt][:, 0:512],
                                 start=(dt == 0), stop=(dt == NDT - 1))
                nc.tensor.matmul(ps_q01[:, 0:512], wq_sb[:, dt, 0:128],
                                 xt_dts[dt][:, 0:512],
                                 start=(dt == 0), stop=(dt == NDT - 1))
                nc.tensor.matmul(ps_q01[:, 512:1024], wq_sb[:, dt, 0:128],
                                 xt_dts[dt][:, 512:1024],
                                 start=(dt == 0), stop=(dt == NDT - 1))
                if dt < NDT - 1:
                    warm(3)
            nc.vector.tensor_scalar_add(out=kTp0[:, 0:512], in0=ps_k0,
                                        scalar1=bk_sb[:, 0:1])
            nc.vector.tensor_scalar_add(out=qTp0[:, 0:1024], in0=ps_q01,
                                        scalar1=bqs_sb[:, 0:1])

            for s, (h, qh) in enumerate(SLABS, start=1):
                attend(h, qh, F[s])

            # ---- tail -------------------------------------------------
            for fn in spill:
                fn()
            # PE transposes (f32r) + DVE copies, all up front: no HWDGE gen
            # on the tail critical path
            for qt in range(8, 16):
                pst = aux_pool.tile([128, 128], f32r, name="pst", tag="aux")
                nc.tensor.matmul(pst, o2q1_sb[:, qt - 8, :], id_sb,
                                 is_transpose=True)
                nc.vector.tensor_copy(out=oT_sb[:, 2, bass.ts(qt, 128)], in_=pst)
            # tail projection: full-width psum rides the freed pss slots;
            # the two yr halves drain on DVE and ACT in parallel
            for qt in range(8, 16):
                # [128, 2, 512]: each nh half owns a full psum bank (matmul
                # accumulation groups must not cross bank boundaries)
                ps = pss_pool.tile([128, 2, 512], f32, name="psyt", tag="pss")
                for nh in range(2):
                    for dtp in range(3):
                        nc.tensor.matmul(
                            ps[:, nh, 0:384], oT_sb[:, dtp, bass.ts(qt, 128)],
                            wp_sb[:, dtp, bass.ts(nh, 384)],
                            start=(dtp == 0), stop=(dtp == 2),
                        )
                yr = yr_pool.tile([128, EMBED], bf16, name="yr", tag="yr")
                nc.vector.tensor_copy(out=yr[:, 0:384], in_=ps[:, 0, 0:384])
                nc.scalar.activation(out=yr[:, 384:768], in_=ps[:, 1, 0:384],
                                     func=ACT_COPY)
                nc.sync.dma_start(out=y_d.ap()[bass.ts(qt, 128), :], in_=yr)

    nc.finalize()
    return nc


def _shard_inputs(x, w_qkv, b_qkv, w_proj):
    import ml_dtypes

    bf16 = ml_dtypes.bfloat16
    in_maps = []
    for c in range(NCORES):
        b, g = c // 2, c % 2
        sl = slice(DL * g, DL * g + DL)
        in_maps.append({
            "xt": np.ascontiguousarray(x[b].T).astype(bf16),
            "wq": np.ascontiguousarray(w_qkv[:, sl] * SCALE).astype(bf16),
            "wk": np.ascontiguousarray(w_qkv[:, EMBED:][:, sl]).astype(bf16),
            "wv": np.ascontiguousarray(w_qkv[:, 2 * EMBED:][:, sl]).astype(bf16),
            "bqs": np.ascontiguousarray(b_qkv[sl] * SCALE),
            "bk": np.ascontiguousarray(b_qkv[EMBED:][sl]),
            "wp": np.ascontiguousarray(w_proj[sl, :]).astype(bf16),
            "ident": np.eye(128, dtype=np.float32),
        })
    return in_maps


def kernel(x, w_qkv, b_qkv, w_proj, b_proj, _profile=False, _repeat=1):
    from concourse.bass_utils import run_bass_kernel_spmd

    x = np.asarray(x, dtype=np.float32)
    w_qkv = np.asarray(w_qkv, dtype=np.float32)
    b_qkv = np.asarray(b_qkv, dtype=np.float32)
    w_proj = np.asarray(w_proj, dtype=np.float32)
    b_proj = np.asarray(b_proj, dtype=np.float32)

    if _repeat not in _prog_cache:
        _prog_cache[_repeat] = _build_program(_repeat)
    nc = _prog_cache[_repeat]

    in_maps = _shard_inputs(x, w_qkv, b_qkv, w_proj)
    res = run_bass_kernel_spmd(
        nc, in_maps, list(range(NCORES)), trace=_profile,
    )

    # host-side gather: sum the two head-group partials per batch and add
    # the bias row (v-bias folded through w_proj, plus b_proj itself)
    bias_row = b_qkv[2 * EMBED:] @ w_proj + b_proj
    y = np.empty((B, T, EMBED), dtype=np.float32)
    for b in range(B):
        y[b] = (res.results[2 * b]["y"].astype(np.float32)
                + res.results[2 * b + 1]["y"].astype(np.float32) + bias_row)
    if _profile:
        return y, res
    return y


# revision 46
# speedup vs baseline: 1.2114x; 1.0183x over previous
"""Multi-head attention (B=4, T=2048, D=768, H=12) on 8 NeuronCores.

Sharding: core c handles batch b = c//2 and head-group g = c%2 (heads
6g..6g+5).  Each core computes its 6 heads' attention and a partial
output projection (contraction over its 384 local dims of w_proj); the
host sums the two partials per batch and adds the bias row.

Device formulation (bf16 matmul operands everywhere, fp32 psum):
  qT = Wq'.T @ xT  [384, 2048]   (Wq' pre-scaled by 1/sqrt(hd) on host)
  kT = Wk.T @ xT   [384, 2048]
  v  = x @ Wv      [2048, 384]   per kpos tile (65th column = 1.0)
  S^T[kt] = kT_h.T @ qT_h   [128 kpos, 1024 q]  per head, q-slab halves
  P^T = exp(S^T)   (ACT; scores max ~8 so no max subtraction)
  O[q, 65] += P^T[kt].T @ v'[kt]   <- flipped: output partitions = 128 q
      (col 64 accumulates the softmax denominators via the ones column)
  o = O[:, 0:64] * rcp(O[:, 64])   fused normalize in the psum drain
  oT via PE transpose (identity stationary), y = oT.T @ Wp per q tile.

The flip halves the P@V matmul cost vs the [65, 512]-output orientation
(the timeline cost model charges out-free-size cycles per matmul, so
output partition utilization is what matters).  Schedule: ACT (exp)
paces the attention inner loop at ~1.04us per [128,1024] tile; S, O,
QKV-projection, V, transposes and the output projection are spread
across the 12 (head, q-slab) sweeps to keep PE under that pace.
O-matmuls trail their exp by 4 kt iterations so psum-slot drains (DVE)
never stall the PE queue head.
"""

import numpy as np

EMBED = 768
HEADS = 12
HD = 64
SCALE = HD ** -0.5
B, T = 4, 2048
NCORES = 8
HPC = 6            # heads per core
DL = HPC * HD      # 384 local model dims per core

_prog_cache = {}


def _build_program(repeat=1):
    import concourse.bass as bass
    import concourse.mybir as mybir
    import concourse.tile as tile
    from concourse import bacc

    f32 = mybir.dt.float32
    f32r = mybir.dt.float32r
    bf16 = mybir.dt.bfloat16
    ACT_EXP = mybir.ActivationFunctionType.Exp
    ACT_COPY = mybir.ActivationFunctionType.Copy

    nc = bacc.Bacc()

    xt_d = nc.dram_tensor("xt", [EMBED, T], bf16, kind="ExternalInput")
    wq_d = nc.dram_tensor("wq", [EMBED, DL], bf16, kind="ExternalInput")
    wk_d = nc.dram_tensor("wk", [EMBED, DL], bf16, kind="ExternalInput")
    wv_d = nc.dram_tensor("wv", [EMBED, DL], bf16, kind="ExternalInput")
    bqs_d = nc.dram_tensor("bqs", [DL], f32, kind="ExternalInput")
    bk_d = nc.dram_tensor("bk", [DL], f32, kind="ExternalInput")
    wp_d = nc.dram_tensor("wp", [DL, EMBED], bf16, kind="ExternalInput")
    id_d = nc.dram_tensor("ident", [128, 128], f32r, kind="ExternalInput")
    y_d = nc.dram_tensor("y", [T, EMBED], bf16, kind="ExternalOutput")

    NDT = EMBED // 128   # 6 contraction tiles over embed dim
    NKT = T // 128       # 16 key-position tiles
    NQT = T // 128       # 16 query row tiles
    LAG = 11             # O-matmul lag (in kt iterations) behind exp: each
    #                      slab's kt5..15 O-matmuls spill into the next slab,
    #                      spreading V/QKV pressure out of the first slabs

    # (head, q-slab) sweep order: q-major within each head pair so a
    # pair's q-half completes as early as possible (feeds transposes).
    SLABS = [(0, 0), (1, 0), (0, 1), (1, 1),
             (2, 0), (3, 0), (2, 1), (3, 1),
             (4, 0), (5, 0), (4, 1), (5, 1)]

    with tile.TileContext(nc) as tc:
      for _rep in range(repeat):
        with tc.tile_pool(name="pers", bufs=1) as pers, \
             tc.tile_pool(name="qk", bufs=2) as qk_pool, \
             tc.tile_pool(name="pt", bufs=2) as pt_pool, \
             tc.tile_pool(name="rcp", bufs=4) as rcp_pool, \
             tc.tile_pool(name="yr", bufs=3) as yr_pool, \
             tc.tile_pool(name="pss", bufs=2, space="PSUM") as pss_pool, \
             tc.tile_pool(name="po", bufs=2, space="PSUM") as po_pool, \
             tc.tile_pool(name="aux", bufs=2, space="PSUM") as aux_pool:

            xt_dts = [pers.tile([128, T], bf16, name=f"xt{dt}_sb")
                      for dt in range(NDT)]
            wq_sb = pers.tile([128, NDT, DL], bf16, name="wq_sb")
            wk_sb = pers.tile([128, NDT, DL], bf16, name="wk_sb")
            wv_sb = pers.tile([128, NDT, DL], bf16, name="wv_sb")
            wp_sb = pers.tile([128, 3, EMBED], bf16, name="wp_sb")
            v_sb = pers.tile([128, NKT, HPC, HD + 1], bf16, name="v_sb")
            bqs_sb = pers.tile([128, 3], f32, name="bqs_sb")
            bk_sb = pers.tile([128, 3], f32, name="bk_sb")
            o_sb = pers.tile([128, NQT, 3, 128], bf16, name="o_sb")
            # pair-2 qh1 stages in f32r so the tail can transpose on the PE
            # (no serial HWDGE descriptor-gens on the critical tail path)
            o2q1_sb = pers.tile([128, 8, 128], f32r, name="o2q1_sb")
            id_sb = pers.tile([128, 128], f32r, name="id_sb")
            oT_sb = pers.tile([128, 3, T], bf16, name="oT_sb")
            warm_sb = pers.tile([128, 256], bf16, name="warm_sb")

            # ones column of v' (softmax denominator accumulator) — only
            # the 65th columns; emitted first so it doesn't sit behind the
            # DMA descriptor generation on the Pool queue
            nc.gpsimd.memset(v_sb[:, :, :, HD:HD + 1], 1.0)
            nc.vector.memset(warm_sb, 0.0)

            # input DMAs: first-slab critical path is wk/wq + all of xt
            # (full embed contraction).  The DMA copies serialize on one
            # resource, so everything not needed before the first S goes
            # after xt.  Per-dt xt tiles give each transfer its own
            # completion sem (DMA write deps are tile x queue granular).
            nc.gpsimd.dma_start(out=wk_sb, in_=wk_d.ap().rearrange("(n p) m -> p n m", p=128))
            nc.gpsimd.dma_start(out=wq_sb, in_=wq_d.ap().rearrange("(n p) m -> p n m", p=128))
            for dt in range(3):
                nc.sync.dma_start(out=xt_dts[dt], in_=xt_d.ap()[bass.ts(dt, 128), :])
            for dt in range(3, NDT):
                nc.gpsimd.dma_start(out=xt_dts[dt], in_=xt_d.ap()[bass.ts(dt, 128), :])
            nc.gpsimd.dma_start(out=wv_sb, in_=wv_d.ap().rearrange("(n p) m -> p n m", p=128))
            nc.gpsimd.dma_start(out=wp_sb, in_=wp_d.ap().rearrange("(n p) m -> p n m", p=128))
            nc.gpsimd.dma_start(out=id_sb, in_=id_d.ap())
            nc.sync.dma_start(out=bqs_sb, in_=bqs_d.ap().rearrange("(n p) -> p n", p=128))
            nc.sync.dma_start(out=bk_sb, in_=bk_d.ap().rearrange("(n p) -> p n", p=128))

            def warm(n):
                # warm-up matmuls ride the po slots (idle until the first
                # slab's O accumulation; pss holds ps_q01 through startup)
                for _w in range(n):
                    psw = po_pool.tile([128, 256], f32, name="psw", tag="po")
                    nc.tensor.matmul(psw, warm_sb[0:2, 0:128], warm_sb[0:2, :],
                                     start=True, stop=True)

            warm(6)

            qk_tiles = {}
            yr_tiles = {}

            def mk_pair(hp):
                qk_tiles[hp] = (
                    qk_pool.tile([128, T], bf16, name="qTp", tag="qT"),
                    qk_pool.tile([128, T], bf16, name="kTp", tag="kT"),
                )

            def qkv_group(hp, ch, which):
                csl = bass.ts(ch, 512)
                qTp, kTp = qk_tiles[hp]
                dst, wsb, bias = (
                    (qTp, wq_sb, bqs_sb) if which == "q" else (kTp, wk_sb, bk_sb)
                )
                ps = aux_pool.tile([128, 512], f32, name="psqk", tag="aux")
                for dt in range(NDT):
                    nc.tensor.matmul(
                        ps, wsb[:, dt, bass.ts(hp, 128)], xt_dts[dt][:, csl],
                        start=(dt == 0), stop=(dt == NDT - 1),
                    )
                nc.vector.tensor_scalar_add(
                    out=dst[:, csl], in0=ps, scalar1=bias[:, hp:hp + 1],
                )

            def v_emit(kt):
                ps = aux_pool.tile([128, DL], f32, name="psv", tag="aux")
                for dt in range(NDT):
                    nc.tensor.matmul(
                        ps, xt_dts[dt][:, bass.ts(kt, 128)], wv_sb[:, dt, :],
                        start=(dt == 0), stop=(dt == NDT - 1),
                    )
                # GPSIMD cannot touch PSUM; DVE is nearly idle during the
                # V-emission slab (ACT copies here would stall its in-order
                # queue ahead of the exps)
                nc.vector.tensor_copy(
                    out=v_sb[:, kt, :, 0:HD],
                    in_=ps.rearrange("p (h d) -> p h d", h=HPC),
                )

            def transpose_qt(pair, qt, tail=False):
                # 2-byte dtypes transpose on the DMA xbar (PE transpose into
                # psum is 4-byte-cell granular and corrupts bf16).  Tail
                # transposes issue from the otherwise-idle ACT queue so their
                # descriptor generation doesn't serialize behind the y DMAs.
                eng = nc.scalar if tail else nc.sync
                eng.dma_start_transpose(
                    out=oT_sb[:, pair, bass.ts(qt, 128)],
                    in_=o_sb[:, qt, pair, :],
                )

            def proj_group(qt, nh, tail=False):
                ps = aux_pool.tile([128, 384], f32, name="psy", tag="aux")
                for dtp in range(3):
                    nc.tensor.matmul(
                        ps, oT_sb[:, dtp, bass.ts(qt, 128)],
                        wp_sb[:, dtp, bass.ts(nh, 384)],
                        start=(dtp == 0), stop=(dtp == 2),
                    )
                if nh == 0:
                    yr = yr_pool.tile([128, EMBED], bf16, name="yr", tag="yr")
                    yr_tiles[qt] = yr
                    nc.vector.tensor_copy(out=yr[:, 0:384], in_=ps)
                else:
                    yr = yr_tiles.pop(qt)
                    if tail:  # ACT is idle once attention has drained
                        nc.scalar.activation(out=yr[:, 384:768], in_=ps, func=ACT_COPY)
                    else:
                        nc.vector.tensor_copy(out=yr[:, 384:768], in_=ps)
                    nc.sync.dma_start(out=y_d.ap()[bass.ts(qt, 128), :], in_=yr)

            def drain_po(h, qh, po, qt_base):
                # fused normalize: o = O[:, 0:64] / O[:, 64] at psum drain
                pair, off = h // 2, (h % 2) * HD
                rcp = rcp_pool.tile([128, 4], f32, name="rcp", tag="rcp")
                nc.vector.reciprocal(out=rcp, in_=po[:, :, HD])
                for j in range(4):
                    qt = qh * 8 + qt_base + j
                    if pair == 2 and qh == 1:
                        dst = o2q1_sb[:, qt - 8, off:off + HD]
                    else:
                        dst = o_sb[:, qt, pair, off:off + HD]
                    with nc.allow_low_precision(reason="f32r staging"):
                        nc.vector.tensor_scalar_mul(
                            out=dst, in0=po[:, j, 0:HD], scalar1=rcp[:, j:j + 1],
                        )

            spill = []   # closures: previous slab's trailing O-matmuls + drains

            def attend(h, qh, fillers, late_fillers=()):
                nonlocal spill
                hp, off = h // 2, (h % 2) * HD
                qTp, kTp = qk_tiles[hp]
                pts = pt_pool.tile([128, NKT, 1024], bf16, name="pts", tag="pt")
                po_t = [None, None]
                myspill = []

                def own_o(kt):
                    for qt in range(8):
                        po = po_t[qt // 4]
                        # start=True zeroes the whole 2KB psum bank, so only
                        # the first column of each po bank may assert it
                        nc.tensor.matmul(
                            po[:, qt % 4, :],
                            pts[:, kt, bass.ts(qt, 128)],
                            v_sb[:, kt, h, :],
                            start=(kt == 0 and qt % 4 == 0),
                            stop=(kt == NKT - 1),
                            skip_group_check=True,
                        )

                # early fillers (V / QKV groups — read no drain-produced
                # tiles) spread over j=0..LAG-1; late fillers (transposes,
                # proj — emission-ordered after the j=9,10 drains) over the
                # remaining iterations
                fi = [0, 0]
                flists = (fillers, late_fillers)
                spans = ((0, LAG), (LAG, NKT))

                def emit_fillers(j, which):
                    lo, hi = spans[which]
                    if j < lo:
                        return
                    fl = flists[which]
                    upto = min(
                        (len(fl) * (j - lo + 1) + (hi - lo) - 1) // (hi - lo),
                        len(fl),
                    )
                    while fi[which] < upto:
                        fl[fi[which]]()
                        fi[which] += 1

                for j in range(NKT):
                    pss = pss_pool.tile([128, 1024], f32, name="pss", tag="pss")
                    for c2 in range(2):
                        nc.tensor.matmul(
                            pss[:, bass.ts(c2, 512)],
                            kTp[off:off + HD, bass.ts(j, 128)],
                            qTp[off:off + HD, bass.ds(qh * 1024 + c2 * 512, 512)],
                            start=True, stop=True,
                        )
                    nc.scalar.activation(out=pts[:, j, :], in_=pss, func=ACT_EXP)
                    if j < len(spill):
                        spill[j]()
                    if j == LAG:
                        po_t[0] = po_pool.tile([128, 4, HD + 1], f32, name="po0", tag="po")
                        po_t[1] = po_pool.tile([128, 4, HD + 1], f32, name="po1", tag="po")
                    if j >= LAG:
                        own_o(j - LAG)
                    emit_fillers(j, 0)
                    emit_fillers(j, 1)

                # trailing O-matmuls kt5..15 spill into the next slab at
                # j=0..8 (two-per-j for the first two), drains at j=9,10 —
                # safely before the po slots are reallocated at j=11
                myspill.append(lambda: (own_o(NKT - LAG), own_o(NKT - LAG + 1)))
                myspill.append(lambda: (own_o(NKT - LAG + 2), own_o(NKT - LAG + 3)))
                for kt in range(NKT - LAG + 4, NKT):
                    myspill.append(lambda kt=kt: own_o(kt))
                myspill.append(lambda: drain_po(h, qh, po_t[0], 0))
                myspill.append(lambda: drain_po(h, qh, po_t[1], 4))
                spill = myspill

            # ---- static filler schedule -------------------------------
            mk_pair(0)
            FE = {s: [] for s in range(1, 13)}   # early: V / QKV groups
            FL = {s: [] for s in range(1, 13)}   # late: transposes / proj
            FE[1] = (
                [lambda: qkv_group(0, 1, "k")]
                + [lambda kt=kt: v_emit(kt) for kt in range(2)]
                + [lambda: qkv_group(0, 2, "k")]
                + [lambda kt=kt: v_emit(kt) for kt in range(2, 4)]
                + [lambda: qkv_group(0, 3, "k")]
                + [lambda kt=kt: v_emit(kt) for kt in range(4, 9)]
            )
            FE[2] = (
                [lambda kt=kt: v_emit(kt) for kt in range(9, 16)]
                + [lambda: qkv_group(0, 2, "q"),
                   lambda: qkv_group(0, 3, "q")]
            )
            FE[3] = [
                lambda: mk_pair(1),
                lambda: qkv_group(1, 0, "k"),
                lambda: qkv_group(1, 0, "q"),
                lambda: qkv_group(1, 1, "k"),
                lambda: qkv_group(1, 1, "q"),
            ]
            FL[3] = [lambda qt=qt: transpose_qt(0, qt) for qt in range(8)]
            FE[4] = [
                lambda: qkv_group(1, 2, "k"),
                lambda: qkv_group(1, 2, "q"),
                lambda: qkv_group(1, 3, "k"),
                lambda: qkv_group(1, 3, "q"),
            ]
            FE[5] = [
                lambda: mk_pair(2),
                lambda: qkv_group(2, 0, "k"),
                lambda: qkv_group(2, 0, "q"),
            ]
            FL[5] = [lambda qt=qt: transpose_qt(0, qt) for qt in range(8, 16)]
            FE[6] = [
                lambda: qkv_group(2, 1, "k"),
                lambda: qkv_group(2, 1, "q"),
            ]
            FL[7] = [lambda qt=qt: transpose_qt(1, qt) for qt in range(8)]
            FE[7] = [
                lambda: qkv_group(2, 2, "k"),
                lambda: qkv_group(2, 2, "q"),
            ]
            FE[8] = [
                lambda: qkv_group(2, 3, "k"),
                lambda: qkv_group(2, 3, "q"),
            ]
            FL[9] = [lambda qt=qt: transpose_qt(1, qt) for qt in range(8, 16)]
            FL[11] = (
                [lambda qt=qt: transpose_qt(2, qt) for qt in range(8)]
                + [lambda qt=qt, nh=nh: proj_group(qt, nh)
                   for qt in range(4) for nh in range(2)]
            )
            FL[12] = [lambda qt=qt, nh=nh: proj_group(qt, nh)
                      for qt in range(4, 8) for nh in range(2)]

            # ---- startup: pair-0 chunks needed by the first slab, with
            # the three accumulations interleaved by dt so each matmul runs
            # as its xt tile lands (q-ch1 borrows a po slot; aux has 2)
            qTp0, kTp0 = qk_tiles[0]
            ps_k0 = aux_pool.tile([128, 512], f32, name="ps_k0", tag="aux")
            ps_q01 = pss_pool.tile([128, 1024], f32, name="ps_q01", tag="pss")
            for dt in range(NDT):
                nc.tensor.matmul(ps_k0, wk_sb[:, dt, 0:128], xt_dts[dt][:, 0:512],
                                 start=(dt == 0), stop=(dt == NDT - 1))
                nc.tensor.matmul(ps_q01[:, 0:512], wq_sb[:, dt, 0:128],
                                 xt_dts[dt][:, 0:512],
                                 start=(dt == 0), stop=(dt == NDT - 1))
                nc.tensor.matmul(ps_q01[:, 512:1024], wq_sb[:, dt, 0:128],
                                 xt_dts[dt][:, 512:1024],
                                 start=(dt == 0), stop=(dt == NDT - 1))
                if dt < NDT - 1:
                    warm(3)
            nc.vector.tensor_scalar_add(out=kTp0[:, 0:512], in0=ps_k0,
                                        scalar1=bk_sb[:, 0:1])
            nc.vector.tensor_scalar_add(out=qTp0[:, 0:1024], in0=ps_q01,
                                        scalar1=bqs_sb[:, 0:1])

            for s, (h, qh) in enumerate(SLABS, start=1):
                attend(h, qh, FE[s], FL[s])

            # ---- tail -------------------------------------------------
            for fn in spill:
                fn()
            # tail: per qt, PE transpose (f32r) + DVE copy feeds the
            # projection immediately (no HWDGE gen on the critical path);
            # full-width proj psum rides the freed pss slots and the two yr
            # halves drain on DVE and ACT in parallel
            for qt in range(8, 16):
                pst = aux_pool.tile([128, 128], f32r, name="pst", tag="aux")
                nc.tensor.matmul(pst, o2q1_sb[:, qt - 8, :], id_sb,
                                 is_transpose=True)
                nc.vector.tensor_copy(out=oT_sb[:, 2, bass.ts(qt, 128)], in_=pst)
                # [128, 2, 512]: each nh half owns a full psum bank (matmul
                # accumulation groups must not cross bank boundaries)
                ps = pss_pool.tile([128, 2, 512], f32, name="psyt", tag="pss")
                for nh in range(2):
                    for dtp in range(3):
                        nc.tensor.matmul(
                            ps[:, nh, 0:384], oT_sb[:, dtp, bass.ts(qt, 128)],
                            wp_sb[:, dtp, bass.ts(nh, 384)],
                            start=(dtp == 0), stop=(dtp == 2),
                        )
                yr = yr_pool.tile([128, EMBED], bf16, name="yr", tag="yr")
                nc.vector.tensor_copy(out=yr[:, 0:384], in_=ps[:, 0, 0:384])
                nc.scalar.activation(out=yr[:, 384:768], in_=ps[:, 1, 0:384],
                                     func=ACT_COPY)
                nc.sync.dma_start(out=y_d.ap()[bass.ts(qt, 128), :], in_=yr)

    nc.finalize()
    return nc


def _shard_inputs(x, w_qkv, b_qkv, w_proj):
    import ml_dtypes

    bf16 = ml_dtypes.bfloat16
    in_maps = []
    for c in range(NCORES):
        b, g = c // 2, c % 2
        sl = slice(DL * g, DL * g + DL)
        in_maps.append({
            "xt": np.ascontiguousarray(x[b].T).astype(bf16),
            "wq": np.ascontiguousarray(w_qkv[:, sl] * SCALE).astype(bf16),
            "wk": np.ascontiguousarray(w_qkv[:, EMBED:][:, sl]).astype(bf16),
            "wv": np.ascontiguousarray(w_qkv[:, 2 * EMBED:][:, sl]).astype(bf16),
            "bqs": np.ascontiguousarray(b_qkv[sl] * SCALE),
            "bk": np.ascontiguousarray(b_qkv[EMBED:][sl]),
            "wp": np.ascontiguousarray(w_proj[sl, :]).astype(bf16),
            "ident": np.eye(128, dtype=np.float32),
        })
    return in_maps


def kernel(x, w_qkv, b_qkv, w_proj, b_proj, _profile=False, _repeat=1):
    from concourse.bass_utils import run_bass_kernel_spmd

    x = np.asarray(x, dtype=np.float32)
    w_qkv = np.asarray(w_qkv, dtype=np.float32)
    b_qkv = np.asarray(b_qkv, dtype=np.float32)
    w_proj = np.asarray(w_proj, dtype=np.float32)
    b_proj = np.asarray(b_proj, dtype=np.float32)

    if _repeat not in _prog_cache:
        _prog_cache[_repeat] = _build_program(_repeat)
    nc = _prog_cache[_repeat]

    in_maps = _shard_inputs(x, w_qkv, b_qkv, w_proj)
    res = run_bass_kernel_spmd(
        nc, in_maps, list(range(NCORES)), trace=_profile,
    )

    # host-side gather: sum the two head-group partials per batch and add
    # the bias row (v-bias folded through w_proj, plus b_proj itself)
    bias_row = b_qkv[2 * EMBED:] @ w_proj + b_proj
    y = np.empty((B, T, EMBED), dtype=np.float32)
    for b in range(B):
        y[b] = (res.results[2 * b]["y"].astype(np.float32)
                + res.results[2 * b + 1]["y"].astype(np.float32) + bias_row)
    if _profile:
        return y, res
    return y
